# revision 64
# baseline (speedup 1.0000x reference)
"""Trainium2 Bass kernel for nn_Attention_28020366639391 (sparse attention).

Math (per batch element b, reference semantics):
    q/k/v = x @ W{q,k,v} + b{q,k,v}, split into 12 heads of 64
    scores = q k^T / 8 ; rows >= 512 zeroed pre-softmax
    -> rows >= 512 have uniform probs = 1/1024 -> ctx row = mean_k(v)
    out = concat_heads(ctx) @ Wo + bo

Sharding: data-parallel on batch. 8 batch elements -> 8 NeuronCores, no
collectives. Each core gets x=[1024,768] + the full weights and computes
out=[1024,768].

Per-core dataflow (v2 — engine-balanced around the PE roofline):
    xT   [768,1024] fp32r = PE-transpose of x (d' on partitions)
    QT/KT bf16 = W{q,k}^T @ xT (Wq/Wk streamed as [768,128] column blocks,
                 2 blocks live at a time)
    V    bf16 per 128-row chunk -> Vaug [k, 12h, 64+ones] (ones column gives
                 softmax row sums for free in the ctx matmul)
    per head pair hp, key chunk kc: scores sT [kc 128, 1024] PSUM tile
                 (head 2hp in cols 0:512, head 2hp+1 in 512:1024),
                 one exp -> e[hp] bf16 [128, kc, 1024] on ACT
    ctx (heads 0-9, "ALT" layout): out [q 128, 65] bf16 matmuls
                 lhsT = e slice [k,128q], rhs = Vaug head slice [k,65];
                 col 64 = softmax denominator. Normalize via per-partition
                 reciprocal + tensor_scalar on DVE -> ctxC fp32r, then PE
                 transposes (lagged one head pair) -> TQ bf16 (d on parts).
    ctx (heads 10-11, "OLD" layout): out [65, 512] keeps d on partitions so
                 the out-projection's last contraction block needs no
                 transpose (shortens the critical tail).
    out rows 0:512   = TQ/ctxT5 blocks @ Wo(bf16)
    out rows 512:1024 = broadcast of (mean_k V) @ Wo (from column sums of x)

All projections run in fp32r (1 cycle/row on the PE); attention ctx runs in
bf16 (needed for 1 cycle/row at free-size 65). Weight tensors stay fp32r in
SBUF except Wo, which is copied to bf16 on GpSimd to match the bf16 ctx
operands (the PE cannot mix 32-bit and 16-bit operands).
"""

import os

import numpy as np

import concourse.bass as bass
import concourse.mybir as mybir
import concourse.tile as tile
from concourse import bacc
from concourse.bass_utils import run_bass_kernel_spmd
from concourse.masks import make_identity

B, S, D, H, DH = 8, 1024, 768, 12, 64
SH = 512            # active (unmasked) query rows = patches//2
DC = D // 128       # 6 chunks of the model dim
SC = S // 128       # 8 chunks of the sequence dim
HP = H // 2         # 6 head pairs
NCORES = 8
FP = mybir.dt.float32
FPR = mybir.dt.float32r
BF = mybir.dt.bfloat16
AF = mybir.ActivationFunctionType
NT = ((0, 512), (512, 256))  # free-dim tiling of a 768-wide output
ALT_HP = HP - 1     # head pairs 0..4 use the ALT (q-partition) ctx layout


def _r(ap):
    """View an fp32 AP as float32r so the PE runs at full rate."""
    return ap.bitcast(FPR)


def _body(tc, out, x, W, bvec, with_bias=True):
    nc = tc.nc
    from contextlib import ExitStack

    with ExitStack() as ctx:
        ctx.enter_context(
            nc.allow_low_precision(reason="fp32r/bf16 PE paths by design")
        )
        constp = ctx.enter_context(tc.tile_pool(name="const", bufs=1))
        wvp = ctx.enter_context(tc.tile_pool(name="wv", bufs=1))
        wop = ctx.enter_context(tc.tile_pool(name="wo", bufs=1))
        wqkp = ctx.enter_context(tc.tile_pool(name="wqk", bufs=2))
        xtp = ctx.enter_context(tc.tile_pool(name="xt", bufs=1))
        qkp = ctx.enter_context(tc.tile_pool(name="qk", bufs=1))
        vap = ctx.enter_context(tc.tile_pool(name="va", bufs=1))
        ep = ctx.enter_context(tc.tile_pool(name="e", bufs=4))
        cxc = ctx.enter_context(tc.tile_pool(name="cxc", bufs=1))
        smallp = ctx.enter_context(tc.tile_pool(name="small", bufs=1))
        op_ = ctx.enter_context(tc.tile_pool(name="o", bufs=2))
        xnp = ctx.enter_context(tc.tile_pool(name="xn", bufs=1))
        # PSUM: scores 2x[128,1024] (4 banks) + projections 1x (2 banks) +
        # two ctx accumulators (1 bank each) = 8 banks exactly
        ssp = ctx.enter_context(tc.tile_pool(name="ss", bufs=2, space="PSUM"))
        pjp = ctx.enter_context(tc.tile_pool(name="pj", bufs=1, space="PSUM"))
        cxp = ctx.enter_context(tc.tile_pool(name="cx", bufs=1, space="PSUM"))

        # ---------------- constants ----------------
        ident = constp.tile([128, 128], FP, tag="ident")
        make_identity(nc, ident[:])
        identR = constp.tile([128, 128], FPR, tag="identR")
        nc.vector.tensor_copy(identR[:], ident[:])
        onesF = constp.tile([1, 512], FP, tag="onesF")
        nc.vector.memset(onesF[:], 1.0)
        ones = constp.tile([1, 512], FPR, tag="ones")
        nc.vector.tensor_copy(ones[:], onesF[:])
        onesB = constp.tile([1, 512], BF, tag="onesB")
        nc.vector.tensor_copy(onesB[:], onesF[:])

        # ---------------- input DMAs, in schedule order ----------------
        # x first (gates everything), then Wq/Wk column blocks just-in-time
        # per head pair, Wv row blocks interleaved, Wo last.
        # x in 4 DMA instructions of 2 row-chunks each (HWDGE descriptor
        # generation serializes at ~625ns/instruction — batch aggressively)
        xn = []
        for g, (r0, nr) in enumerate(((0, 2), (2, 2), (4, 2), (6, 2))):
            t = xnp.tile([128, 2, D], FPR, tag=f"xh{g % 2}", name=f"xn{g}")
            nc.sync.dma_start(
                out=t[:, 0:nr, :],
                in_=_r(x[r0 * 128:(r0 + nr) * 128, :]).rearrange(
                    "(s p) d -> p s d", p=128
                ),
            )
            xn.append((t, r0, nr))

        wq = []
        wk = []

        def load_wqk(c):
            for nm, lst in (("Wq", wq), ("Wk", wk)):
                t = wqkp.tile([128, DC, 128], FPR, tag=nm, name=f"{nm}{c}")
                nc.sync.dma_start(
                    out=t[:],
                    in_=_r(W[nm][:, c * 128:(c + 1) * 128]).rearrange(
                        "(k p) c -> p k c", p=128
                    ),
                )
                lst.append(t)

        wv = wvp.tile([128, DC, D], FPR, tag="Wv", name="w_Wv")

        load_wqk(0)
        load_wqk(1)
        nc.sync.dma_start(
            out=wv[:], in_=_r(W["Wv"][:, :]).rearrange("(k p) d -> p k d", p=128)
        )
        load_wqk(2)
        load_wqk(3)
        load_wqk(4)
        load_wqk(5)

        # Wo: stage fp32 in the x slots, convert to bf16 on GpSimd
        wob = wop.tile([128, DC, D], BF, tag="Wob", name="w_Wob")
        for g in range(3):
            wos = xnp.tile([128, 2, D], FPR, tag=f"xh{g % 2}", name=f"wos{g}")
            nc.sync.dma_start(
                out=wos[:],
                in_=_r(W["Wo"][g * 256:(g + 1) * 256, :]).rearrange(
                    "(k p) d -> p k d", p=128
                ),
            )
            nc.gpsimd.tensor_copy(wob[:, 2 * g:2 * g + 2, :], wos[:].bitcast(FP))

        brow = {}
        if with_bias:
            for nm in ("bq", "bk", "bv", "bo"):
                t = constp.tile([1, D], FPR, tag=f"brow_{nm}", name=f"brow_{nm}")
                nc.sync.dma_start(out=t[:], in_=_r(bvec[nm][None, :]))
                brow[nm] = t
            bvT = constp.tile([128, DC], FP, tag="bvT")
            for c in range(DC):
                nc.sync.dma_start(
                    out=bvT[:, c:c + 1], in_=bvec["bv"][c * 128:(c + 1) * 128, None]
                )
            browB = constp.tile([1, D], BF, tag="browB")
            nc.vector.tensor_copy(browB[:], brow["bo"][:].bitcast(FP))

        # ---------------- persistent tensors ----------------
        xT = xtp.tile([128, DC, S], FPR, tag="xT")
        QT = qkp.tile([128, DC, SH], BF, tag="QT")
        KT = qkp.tile([128, DC, S], BF, tag="KT")
        Vaug = vap.tile([128, SC, H, DH + 1], BF, tag="Vaug")
        vones = constp.tile([128, SC * H], FP, tag="vones")
        nc.vector.memset(vones[:], 1.0)
        nc.vector.tensor_copy(
            Vaug[:, :, :, DH:DH + 1],
            vones[:].rearrange("p (k h) -> p k h", k=SC)[:, :, :, None],
        )
        # ctxC: per head pair, 512 columns interleaved as (d_local, qc) ->
        # d_local*4 + qc, so ONE xbar transpose per head pair yields
        # TQ[hp][p, qc, q] = ctxT[d = hp*128 + p, qc*128 + q]
        ctxC = cxc.tile([128, ALT_HP, 512], FPR, tag="ctxC")
        TQ = [cxc.tile([128, 4, 128], BF, tag=f"TQ{hp}", name=f"TQ{hp}")
              for hp in range(ALT_HP)]
        ctxT5 = cxc.tile([128, SH], BF, tag="ctxT5")
        xsum = constp.tile([128, DC], FPR, tag="xsum")
        mvt = constp.tile([128, DC], BF, tag="mvt")
        ttile = constp.tile([128, D], FP, tag="ttile")

        # ---------------- x transposes (PE, fp32r: 1.5 cycles/row) --------
        for t, r0, nr in xn:
            for s in range(nr):
                sc = r0 + s
                pt = ssp.tile([128, 1024], FP, tag="ss", name=f"ptx{sc}")
                for c in range(DC):
                    nc.tensor.transpose(
                        _r(pt[:, c * 128:(c + 1) * 128]),
                        t[:, s, c * 128:(c + 1) * 128],
                        identR[:],
                    )
                ptv = pt[:, 0:D].rearrange("p (c s) -> p c s", c=DC)
                nc.vector.tensor_copy(xT[:, :, sc * 128:(sc + 1) * 128], _r(ptv))

        # ------------- quantum generators (PE filler interleaving) -------
        # The exp stream on ACT (~1038ns per [128,1024] tile) is 2.4x slower
        # than the two score matmuls feeding it (~426ns). s_chunk emits
        # filler quanta of independent PE work between score pairs so the PE
        # never throttles to the ACT rate.
        from collections import deque

        fillers = deque()

        def take_filler(n):
            done = 0
            while done < n and fillers:
                try:
                    next(fillers[0])
                    done += 1
                except StopIteration:
                    fillers.popleft()

        def drain_fillers():
            while fillers:
                try:
                    next(fillers[0])
                except StopIteration:
                    fillers.popleft()

        def qkt_gen(c):
            """QT/KT projection chunk c in ~2-matmul quanta."""
            pq = pjp.tile([128, 1024], FP, tag="pj", name=f"pq{c}")
            for k in range(DC):
                nc.tensor.matmul(
                    pq[:, 0:SH],
                    wq[c][:, k, :],
                    xT[:, k, 0:SH],
                    start=(k == 0),
                    stop=(not with_bias and k == DC - 1),
                )
                if k % 2 == 1:
                    yield
            if with_bias:
                nc.tensor.matmul(
                    pq[:, 0:SH],
                    _r(brow["bq"][0:1, c * 128:(c + 1) * 128]),
                    ones[0:1, 0:SH],
                    start=False,
                    stop=True,
                )
            nc.vector.tensor_copy(QT[:, c, :], pq[:, 0:SH])
            yield
            pk = pjp.tile([128, 1024], FP, tag="pj", name=f"pk{c}")
            for sg in range(2):
                for k in range(DC):
                    nc.tensor.matmul(
                        pk[:, sg * 512:sg * 512 + 512],
                        wk[c][:, k, :],
                        xT[:, k, sg * 512:sg * 512 + 512],
                        start=(k == 0),
                        stop=(not with_bias and k == DC - 1),
                    )
                    if k % 2 == 1:
                        yield
                if with_bias:
                    nc.tensor.matmul(
                        pk[:, sg * 512:sg * 512 + 512],
                        _r(brow["bk"][0:1, c * 128:(c + 1) * 128]),
                        ones[0:1, 0:512],
                        start=False,
                        stop=True,
                    )
                nc.vector.tensor_copy(
                    KT[:, c, sg * 512:sg * 512 + 512],
                    pk[:, sg * 512:sg * 512 + 512],
                )
                yield

        def v_gen(sc):
            """V projection chunk sc in ~2-matmul quanta."""
            pv = pjp.tile([128, 1024], FP, tag="pj", name=f"pv{sc}")
            for n0, nw in NT:
                for k in range(DC):
                    nc.tensor.matmul(
                        pv[:, n0:n0 + nw],
                        xT[:, k, sc * 128:(sc + 1) * 128],
                        wv[:, k, n0:n0 + nw],
                        start=(k == 0),
                        stop=(not with_bias and k == DC - 1),
                    )
                    if k % 2 == 1:
                        yield
                if with_bias:
                    nc.tensor.matmul(
                        pv[:, n0:n0 + nw],
                        ones[0:1, 0:128],
                        _r(brow["bv"][0:1, n0:n0 + nw]),
                        start=False,
                        stop=True,
                    )
            nc.vector.tensor_copy(
                Vaug[:, sc, :, 0:DH],
                pv[:, 0:D].rearrange("p (h e) -> p h e", h=H),
            )
            yield

        def s_chunk(hp, take=2, must=None):
            """Scores + exp for head pair hp; returns the e tile.

            `must` is the generator producing this head pair's QT/KT — it is
            drained first so every score matmul reads completed copies
            (emission order defines the dataflow graph)."""
            if must is not None:
                for _ in must:
                    pass
            e = ep.tile([128, SC, 1024], BF, tag="e", name=f"e{hp}")
            for kc in range(SC):
                st = ssp.tile([128, 1024], FP, tag="ss", name=f"st{hp}_{kc}")
                for j in range(2):
                    off = j * DH
                    nc.tensor.matmul(
                        st[:, j * 512:j * 512 + 512],
                        KT[off:off + DH, hp, kc * 128:(kc + 1) * 128],
                        QT[off:off + DH, hp, :],
                        start=True,
                        stop=True,
                        tile_position=(off, 0),
                    )
                nc.scalar.activation(e[:, kc, :], st[:], AF.Exp, scale=0.125)
                take_filler(take)
            return e

        def tq_part(hp):
            """PE-transpose ctxC[hp] -> TQ[hp] (the xbar DMA transpose's
            completion semaphore is unreliable for PE consumers in real
            execution, so stay on the well-trodden matmul path)."""
            ptq = pjp.tile([128, 1024], FP, tag="pj", name=f"ptq{hp}")
            for qc in range(4):
                nc.tensor.transpose(
                    _r(ptq[:, qc * 128:(qc + 1) * 128]),
                    ctxC[:, hp, qc * 128:(qc + 1) * 128],
                    identR[:],
                )
            nc.vector.tensor_copy(
                TQ[hp][:].rearrange("p a b -> p (a b)"), _r(ptq[:, 0:512])
            )

        def ctx_norm(hp, e):
            """ALT-layout ctx + normalization for head pair hp."""
            rec = smallp.tile([128, 2, 4], FP, tag="rec", name=f"rec{hp}")
            for j, eng in ((0, nc.vector), (1, nc.vector)):
                pc = cxp.tile([128, 4, DH + 1], FP, tag=f"cx{j}", name=f"cx{hp}_{j}")
                h = 2 * hp + j
                for qc in range(4):
                    for kc in range(SC):
                        nc.tensor.matmul(
                            pc[:, qc, :],
                            e[:, kc, j * 512 + qc * 128:j * 512 + qc * 128 + 128],
                            Vaug[:, kc, h, :],
                            start=(kc == 0),
                            stop=(kc == SC - 1),
                        )
                nc.vector.reciprocal(rec[:, j, :], pc[:, :, DH:DH + 1])
                for qc in range(4):
                    # ctxC col layout qc*128 + d_local: the xbar transpose's
                    # 3D out [128, 4, 128] maps out[p, f1, f2] = in[f2, f1*128+p]
                    eng.tensor_scalar_mul(
                        ctxC[:, hp, qc * 128 + j * DH:qc * 128 + (j + 1) * DH],
                        pc[:, qc, 0:DH],
                        rec[:, j, qc:qc + 1],
                    )

        def meanv_gen():
            """Masked-row tail: mean_k(V) @ Wo + broadcast rows 512:1024."""
            pm = pjp.tile([128, 1024], FP, tag="pj", name="pm")
            for c in range(DC):
                for k in range(DC):
                    nc.tensor.matmul(
                        pm[:, c:c + 1],
                        wv[:, k, c * 128:(c + 1) * 128].bitcast(FP),
                        xsum[:, k:k + 1].bitcast(FP),

                        start=(k == 0),
                        stop=(k == DC - 1),
                    )
                if c % 2 == 1:
                    yield
            if with_bias:
                nc.vector.scalar_tensor_tensor(
                    mvt[:], pm[:, 0:DC], 1.0 / S,
                    bvT[:], op0=mybir.AluOpType.mult, op1=mybir.AluOpType.add,
                )
            else:
                nc.vector.tensor_scalar_mul(mvt[:], pm[:, 0:DC], 1.0 / S)
            yield
            pt2 = pjp.tile([128, 1024], FP, tag="pj", name="pt2")
            for n0, nw in NT:
                for k in range(DC):
                    nc.tensor.matmul(
                        pt2[0:1, n0:n0 + nw],
                        mvt[:, k:k + 1],
                        wob[:, k, n0:n0 + nw],
                        start=(k == 0),
                        stop=(not with_bias and k == DC - 1),
                    )
                    if k % 2 == 1:
                        yield
                if with_bias:
                    nc.tensor.matmul(
                        pt2[0:1, n0:n0 + nw],
                        onesB[0:1, 0:1],
                        browB[0:1, n0:n0 + nw],
                        start=False,
                        stop=True,
                    )
            trow = constp.tile([1, D], FP, tag="trow")
            nc.vector.tensor_copy(trow[:], pt2[0:1, 0:D])
            nc.gpsimd.partition_broadcast(ttile[:], trow[0:1, :])
            for sc in range(SH // 128, SC):
                nc.sync.dma_start(
                    out=out[sc * 128:(sc + 1) * 128, :], in_=ttile[:]
                )
            yield

        def ctx_old(hp, e):
            """OLD-layout ctx for the last head pair: ctxT5 [128 d, 512 q].
            Both heads share one PSUM tile (regions 0:512 / 512:1024)."""
            p5 = pjp.tile([128, 1024], FP, tag="pj", name="p5")
            for j in range(2):
                h = 2 * hp + j
                for kc in range(SC):
                    nc.tensor.matmul(
                        p5[0:DH + 1, j * 512:j * 512 + 512],
                        Vaug[:, kc, h, :],
                        e[:, kc, j * 512:j * 512 + 512],
                        start=(kc == 0),
                        stop=(kc == SC - 1),
                    )
            for j in range(2):
                recrow = smallp.tile([1, SH], FP, tag="recrow", name=f"rr{j}")
                nc.vector.reciprocal(recrow[:], p5[DH:DH + 1, j * 512:j * 512 + 512])
                bsb = smallp.tile([DH, SH], FP, tag="bsb", name=f"bsb{j}")
                nc.gpsimd.partition_broadcast(bsb[:], recrow[0:1, :])
                nc.vector.tensor_mul(
                    ctxT5[j * DH:(j + 1) * DH, :],
                    p5[0:DH, j * 512:j * 512 + 512],
                    bsb[:],
                )

        # ---------------- main schedule ----------------
        # drain one generator fully
        def run_all(g):
            for _ in g:
                pass

        run_all(qkt_gen(0))
        g = [qkt_gen(c) for c in range(1, DC)]
        fillers.append(g[0])
        es = [s_chunk(0, take=4)]
        fillers.append(g[1])
        fillers.append(v_gen(0))
        fillers.append(v_gen(1))
        es.append(s_chunk(1, take=4, must=g[0]))
        fillers.append(v_gen(2))
        fillers.append(v_gen(3))
        fillers.append(v_gen(4))
        fillers.append(g[2])
        es.append(s_chunk(2, take=4, must=g[1]))
        fillers.append(v_gen(5))
        fillers.append(v_gen(6))
        fillers.append(v_gen(7))
        fillers.append(g[3])
        es.append(s_chunk(3, take=4, must=g[2]))
        drain_fillers()
        ctx_norm(0, es[0])
        fillers.append(g[4])
        es.append(s_chunk(4, take=4, must=g[3]))
        drain_fillers()
        # column sums of x for the masked-row tail (after the qkt5 copies so
        # it does not block them on the in-order DVE queue)
        for c in range(DC):
            nc.vector.tensor_reduce(
                xsum[:, c:c + 1, None], xT[:, c, :].bitcast(FP),
                axis=mybir.AxisListType.X, op=mybir.AluOpType.add,
            )
        ctx_norm(1, es[1])
        tq_part(0)
        fillers.append(meanv_gen())
        es.append(s_chunk(5, take=4, must=g[4]))
        drain_fillers()
        ctx_norm(2, es[2])
        tq_part(1)
        ctx_norm(3, es[3])
        tq_part(2)
        ctx_norm(4, es[4])
        tq_part(3)

        tq_part(4)
        # last head pair in the OLD layout (no transpose on the tail)
        ctx_old(ALT_HP, es[ALT_HP])

        # ---------------- output projection, rows 0:512 ----------------
        # qc 0/1: accumulate the TQ blocks (dc 0..4) before the exp-gated
        # ctx_old, close with the ctxT5 block after; qc 2/3 run whole
        for qc in range(4):
            po = ssp.tile([128, 1024], FP, tag="ss", name=f"po{qc}")
            osb = op_.tile([128, D], FP, tag="osb", name=f"osb{qc}")
            for n0, nw in NT:
                for dc in range(ALT_HP):
                    nc.tensor.matmul(
                        po[:, n0:n0 + nw],
                        TQ[dc][:, qc, :],
                        wob[:, dc, n0:n0 + nw],
                        start=(dc == 0),
                        stop=False,
                    )
                nc.tensor.matmul(
                    po[:, n0:n0 + nw],
                    ctxT5[:, qc * 128:(qc + 1) * 128],
                    wob[:, ALT_HP, n0:n0 + nw],
                    start=False,
                    stop=(not with_bias),
                )
                if with_bias:
                    nc.tensor.matmul(
                        po[:, n0:n0 + nw],
                        onesB[0:1, 0:128],
                        browB[0:1, n0:n0 + nw],
                        start=False,
                        stop=True,
                    )
                nc.vector.tensor_copy(osb[:, n0:n0 + nw], po[:, n0:n0 + nw])
                nc.sync.dma_start(
                    out=out[qc * 128:(qc + 1) * 128, n0:n0 + nw],
                    in_=osb[:, n0:n0 + nw],
                )

def build_nc(with_bias=True):
    nc = bacc.Bacc("TRN2", target_bir_lowering=False, debug=False, num_devices=NCORES)
    x = nc.dram_tensor("x", [S, D], FP, kind="ExternalInput").ap()
    W = {
        nm: nc.dram_tensor(nm, [D, D], FP, kind="ExternalInput").ap()
        for nm in ("Wq", "Wk", "Wv", "Wo")
    }
    bvec = {
        nm: nc.dram_tensor(nm, [D], FP, kind="ExternalInput").ap()
        for nm in ("bq", "bk", "bv", "bo")
    }
    out = nc.dram_tensor("out", [S, D], FP, kind="ExternalOutput").ap()
    with tile.TileContext(nc) as tc:
        _body(tc, out, x, W, bvec, with_bias=with_bias)
    nc.compile()
    return nc


def kernel(hidden_states, Wq, bq, Wk, bk, Wv, bv, Wo, bo, _trace=False):
    hidden_states = np.ascontiguousarray(np.asarray(hidden_states, dtype=np.float32))
    shared = {
        "Wq": np.ascontiguousarray(np.asarray(Wq, np.float32)),
        "Wk": np.ascontiguousarray(np.asarray(Wk, np.float32)),
        "Wv": np.ascontiguousarray(np.asarray(Wv, np.float32)),
        "Wo": np.ascontiguousarray(np.asarray(Wo, np.float32)),
        "bq": np.ascontiguousarray(np.asarray(bq, np.float32)),
        "bk": np.ascontiguousarray(np.asarray(bk, np.float32)),
        "bv": np.ascontiguousarray(np.asarray(bv, np.float32)),
        "bo": np.ascontiguousarray(np.asarray(bo, np.float32)),
    }
    with_bias = any(np.any(shared[b]) for b in ("bq", "bk", "bv", "bo"))
    nc = build_nc(with_bias=with_bias)
    in_maps = [{"x": hidden_states[i], **shared} for i in range(NCORES)]
    res = run_bass_kernel_spmd(
        nc, in_maps, core_ids=list(range(NCORES)), trace=_trace
    )
    out = np.stack([res.results[i]["out"] for i in range(NCORES)], axis=0)
    if _trace:
        kernel.last_results = res
    return out


if __name__ == "__main__":
    rng = np.random.default_rng(0)
    ins = {
        "hidden_states": rng.standard_normal((B, S, D), dtype=np.float32),
        **{w: (rng.standard_normal((D, D)) / np.sqrt(D)).astype(np.float32) for w in ("Wq", "Wk", "Wv", "Wo")},
        **{b: np.zeros(D, np.float32) for b in ("bq", "bk", "bv", "bo")},
    }
    o = kernel(**ins)
    print("kernel ran, out shape", o.shape)


# revision 65
# speedup vs baseline: 1.0045x; 1.0045x over previous
"""Trainium2 Bass kernel for nn_Attention_28020366639391 (sparse attention).

Math (per batch element b, reference semantics):
    q/k/v = x @ W{q,k,v} + b{q,k,v}, split into 12 heads of 64
    scores = q k^T / 8 ; rows >= 512 zeroed pre-softmax
    -> rows >= 512 have uniform probs = 1/1024 -> ctx row = mean_k(v)
    out = concat_heads(ctx) @ Wo + bo

Sharding: data-parallel on batch. 8 batch elements -> 8 NeuronCores, no
collectives. Each core gets x=[1024,768] + the full weights and computes
out=[1024,768].

Per-core dataflow (v2 — engine-balanced around the PE roofline):
    xT   [768,1024] fp32r = PE-transpose of x (d' on partitions)
    QT/KT bf16 = W{q,k}^T @ xT (Wq/Wk streamed as [768,128] column blocks,
                 2 blocks live at a time)
    V    bf16 per 128-row chunk -> Vaug [k, 12h, 64+ones] (ones column gives
                 softmax row sums for free in the ctx matmul)
    per head pair hp, key chunk kc: scores sT [kc 128, 1024] PSUM tile
                 (head 2hp in cols 0:512, head 2hp+1 in 512:1024),
                 one exp -> e[hp] bf16 [128, kc, 1024] on ACT
    ctx (heads 0-9, "ALT" layout): out [q 128, 65] bf16 matmuls
                 lhsT = e slice [k,128q], rhs = Vaug head slice [k,65];
                 col 64 = softmax denominator. Normalize via per-partition
                 reciprocal + tensor_scalar on DVE -> ctxC fp32r, then PE
                 transposes (lagged one head pair) -> TQ bf16 (d on parts).
    ctx (heads 10-11, "OLD" layout): out [65, 512] keeps d on partitions so
                 the out-projection's last contraction block needs no
                 transpose (shortens the critical tail).
    out rows 0:512   = TQ/ctxT5 blocks @ Wo(bf16)
    out rows 512:1024 = broadcast of (mean_k V) @ Wo (from column sums of x)

All projections run in fp32r (1 cycle/row on the PE); attention ctx runs in
bf16 (needed for 1 cycle/row at free-size 65). Weight tensors stay fp32r in
SBUF except Wo, which is copied to bf16 on GpSimd to match the bf16 ctx
operands (the PE cannot mix 32-bit and 16-bit operands).
"""

import os

import numpy as np

import concourse.bass as bass
import concourse.mybir as mybir
import concourse.tile as tile
from concourse import bacc
from concourse.bass_utils import run_bass_kernel_spmd
from concourse.masks import make_identity

B, S, D, H, DH = 8, 1024, 768, 12, 64
SH = 512            # active (unmasked) query rows = patches//2
DC = D // 128       # 6 chunks of the model dim
SC = S // 128       # 8 chunks of the sequence dim
HP = H // 2         # 6 head pairs
NCORES = 8
FP = mybir.dt.float32
FPR = mybir.dt.float32r
BF = mybir.dt.bfloat16
AF = mybir.ActivationFunctionType
NT = ((0, 512), (512, 256))  # free-dim tiling of a 768-wide output
ALT_HP = HP - 1     # head pairs 0..4 use the ALT (q-partition) ctx layout


def _r(ap):
    """View an fp32 AP as float32r so the PE runs at full rate."""
    return ap.bitcast(FPR)


def _body(tc, out, x, W, bvec, with_bias=True):
    nc = tc.nc
    from contextlib import ExitStack

    with ExitStack() as ctx:
        ctx.enter_context(
            nc.allow_low_precision(reason="fp32r/bf16 PE paths by design")
        )
        constp = ctx.enter_context(tc.tile_pool(name="const", bufs=1))
        wvp = ctx.enter_context(tc.tile_pool(name="wv", bufs=1))
        wop = ctx.enter_context(tc.tile_pool(name="wo", bufs=1))
        wqkp = ctx.enter_context(tc.tile_pool(name="wqk", bufs=2))
        xtp = ctx.enter_context(tc.tile_pool(name="xt", bufs=1))
        qkp = ctx.enter_context(tc.tile_pool(name="qk", bufs=1))
        vap = ctx.enter_context(tc.tile_pool(name="va", bufs=1))
        ep = ctx.enter_context(tc.tile_pool(name="e", bufs=4))
        cxc = ctx.enter_context(tc.tile_pool(name="cxc", bufs=1))
        smallp = ctx.enter_context(tc.tile_pool(name="small", bufs=1))
        op_ = ctx.enter_context(tc.tile_pool(name="o", bufs=2))
        xnp = ctx.enter_context(tc.tile_pool(name="xn", bufs=1))
        # PSUM: scores 2x[128,1024] (4 banks) + projections 1x (2 banks) +
        # two ctx accumulators (1 bank each) = 8 banks exactly
        ssp = ctx.enter_context(tc.tile_pool(name="ss", bufs=2, space="PSUM"))
        pjp = ctx.enter_context(tc.tile_pool(name="pj", bufs=1, space="PSUM"))
        cxp = ctx.enter_context(tc.tile_pool(name="cx", bufs=1, space="PSUM"))

        # ---------------- constants ----------------
        ident = constp.tile([128, 128], FP, tag="ident")
        make_identity(nc, ident[:])
        identR = constp.tile([128, 128], FPR, tag="identR")
        nc.vector.tensor_copy(identR[:], ident[:])
        onesF = constp.tile([1, 512], FP, tag="onesF")
        nc.vector.memset(onesF[:], 1.0)
        ones = constp.tile([1, 512], FPR, tag="ones")
        nc.vector.tensor_copy(ones[:], onesF[:])
        onesB = constp.tile([1, 512], BF, tag="onesB")
        nc.vector.tensor_copy(onesB[:], onesF[:])

        # ---------------- input DMAs, in schedule order ----------------
        # x first (gates everything), then Wq/Wk column blocks just-in-time
        # per head pair, Wv row blocks interleaved, Wo last.
        # x in 4 DMA instructions of 2 row-chunks each (HWDGE descriptor
        # generation serializes at ~625ns/instruction — batch aggressively)
        xn = []
        for g, (r0, nr) in enumerate(((0, 2), (2, 2), (4, 2), (6, 2))):
            t = xnp.tile([128, 2, D], FPR, tag=f"xh{g % 2}", name=f"xn{g}")
            nc.sync.dma_start(
                out=t[:, 0:nr, :],
                in_=_r(x[r0 * 128:(r0 + nr) * 128, :]).rearrange(
                    "(s p) d -> p s d", p=128
                ),
            )
            xn.append((t, r0, nr))

        wq = []
        wk = []

        def load_wqk(c):
            for nm, lst in (("Wq", wq), ("Wk", wk)):
                t = wqkp.tile([128, DC, 128], FPR, tag=nm, name=f"{nm}{c}")
                nc.sync.dma_start(
                    out=t[:],
                    in_=_r(W[nm][:, c * 128:(c + 1) * 128]).rearrange(
                        "(k p) c -> p k c", p=128
                    ),
                )
                lst.append(t)

        wv = wvp.tile([128, DC, D], FPR, tag="Wv", name="w_Wv")

        load_wqk(0)
        load_wqk(1)
        nc.sync.dma_start(
            out=wv[:], in_=_r(W["Wv"][:, :]).rearrange("(k p) d -> p k d", p=128)
        )
        load_wqk(2)
        load_wqk(3)
        load_wqk(4)
        load_wqk(5)

        # Wo: stage fp32 in the x slots, convert to bf16 on GpSimd
        wob = wop.tile([128, DC, D], BF, tag="Wob", name="w_Wob")
        for g in range(3):
            wos = xnp.tile([128, 2, D], FPR, tag=f"xh{g % 2}", name=f"wos{g}")
            nc.sync.dma_start(
                out=wos[:],
                in_=_r(W["Wo"][g * 256:(g + 1) * 256, :]).rearrange(
                    "(k p) d -> p k d", p=128
                ),
            )
            nc.gpsimd.tensor_copy(wob[:, 2 * g:2 * g + 2, :], wos[:].bitcast(FP))

        brow = {}
        if with_bias:
            for nm in ("bq", "bk", "bv", "bo"):
                t = constp.tile([1, D], FPR, tag=f"brow_{nm}", name=f"brow_{nm}")
                nc.sync.dma_start(out=t[:], in_=_r(bvec[nm][None, :]))
                brow[nm] = t
            bvT = constp.tile([128, DC], FP, tag="bvT")
            for c in range(DC):
                nc.sync.dma_start(
                    out=bvT[:, c:c + 1], in_=bvec["bv"][c * 128:(c + 1) * 128, None]
                )
            browB = constp.tile([1, D], BF, tag="browB")
            nc.vector.tensor_copy(browB[:], brow["bo"][:].bitcast(FP))

        # ---------------- persistent tensors ----------------
        xT = xtp.tile([128, DC, S], FPR, tag="xT")
        QT = qkp.tile([128, DC, SH], BF, tag="QT")
        KT = qkp.tile([128, DC, S], BF, tag="KT")
        Vaug = vap.tile([128, SC, H, DH + 1], BF, tag="Vaug")
        vones = constp.tile([128, SC * H], FP, tag="vones")
        nc.vector.memset(vones[:], 1.0)
        nc.vector.tensor_copy(
            Vaug[:, :, :, DH:DH + 1],
            vones[:].rearrange("p (k h) -> p k h", k=SC)[:, :, :, None],
        )
        # ctxC: per head pair, 512 columns interleaved as (d_local, qc) ->
        # d_local*4 + qc, so ONE xbar transpose per head pair yields
        # TQ[hp][p, qc, q] = ctxT[d = hp*128 + p, qc*128 + q]
        ctxC = cxc.tile([128, ALT_HP, 512], FPR, tag="ctxC")
        TQ = [cxc.tile([128, 4, 128], BF, tag=f"TQ{hp}", name=f"TQ{hp}")
              for hp in range(ALT_HP)]
        ctxT5 = cxc.tile([128, SH], BF, tag="ctxT5")
        xsum = constp.tile([128, DC], FPR, tag="xsum")
        mvt = constp.tile([128, DC], BF, tag="mvt")
        ttile = constp.tile([128, D], FP, tag="ttile")

        # ---------------- x transposes (PE, fp32r: 1.5 cycles/row) --------
        for t, r0, nr in xn:
            for s in range(nr):
                sc = r0 + s
                pt = ssp.tile([128, 1024], FP, tag="ss", name=f"ptx{sc}")
                for c in range(DC):
                    nc.tensor.transpose(
                        _r(pt[:, c * 128:(c + 1) * 128]),
                        t[:, s, c * 128:(c + 1) * 128],
                        identR[:],
                    )
                ptv = pt[:, 0:D].rearrange("p (c s) -> p c s", c=DC)
                nc.vector.tensor_copy(xT[:, :, sc * 128:(sc + 1) * 128], _r(ptv))

        # ------------- quantum generators (PE filler interleaving) -------
        # The exp stream on ACT (~1038ns per [128,1024] tile) is 2.4x slower
        # than the two score matmuls feeding it (~426ns). s_chunk emits
        # filler quanta of independent PE work between score pairs so the PE
        # never throttles to the ACT rate.
        from collections import deque

        fillers = deque()

        def take_filler(n):
            done = 0
            while done < n and fillers:
                try:
                    next(fillers[0])
                    done += 1
                except StopIteration:
                    fillers.popleft()

        def drain_fillers():
            while fillers:
                try:
                    next(fillers[0])
                except StopIteration:
                    fillers.popleft()

        def qkt_gen(c):
            """QT/KT projection chunk c in ~2-matmul quanta."""
            pq = pjp.tile([128, 1024], FP, tag="pj", name=f"pq{c}")
            for k in range(DC):
                nc.tensor.matmul(
                    pq[:, 0:SH],
                    wq[c][:, k, :],
                    xT[:, k, 0:SH],
                    start=(k == 0),
                    stop=(not with_bias and k == DC - 1),
                )
                if k % 2 == 1:
                    yield
            if with_bias:
                nc.tensor.matmul(
                    pq[:, 0:SH],
                    _r(brow["bq"][0:1, c * 128:(c + 1) * 128]),
                    ones[0:1, 0:SH],
                    start=False,
                    stop=True,
                )
            nc.vector.tensor_copy(QT[:, c, :], pq[:, 0:SH])
            yield
            pk = pjp.tile([128, 1024], FP, tag="pj", name=f"pk{c}")
            for sg in range(2):
                for k in range(DC):
                    nc.tensor.matmul(
                        pk[:, sg * 512:sg * 512 + 512],
                        wk[c][:, k, :],
                        xT[:, k, sg * 512:sg * 512 + 512],
                        start=(k == 0),
                        stop=(not with_bias and k == DC - 1),
                    )
                    if k % 2 == 1:
                        yield
                if with_bias:
                    nc.tensor.matmul(
                        pk[:, sg * 512:sg * 512 + 512],
                        _r(brow["bk"][0:1, c * 128:(c + 1) * 128]),
                        ones[0:1, 0:512],
                        start=False,
                        stop=True,
                    )
                nc.vector.tensor_copy(
                    KT[:, c, sg * 512:sg * 512 + 512],
                    pk[:, sg * 512:sg * 512 + 512],
                )
                yield

        def v_gen(sc):
            """V projection chunk sc in ~2-matmul quanta."""
            pv = pjp.tile([128, 1024], FP, tag="pj", name=f"pv{sc}")
            for n0, nw in NT:
                for k in range(DC):
                    nc.tensor.matmul(
                        pv[:, n0:n0 + nw],
                        xT[:, k, sc * 128:(sc + 1) * 128],
                        wv[:, k, n0:n0 + nw],
                        start=(k == 0),
                        stop=(not with_bias and k == DC - 1),
                    )
                    if k % 2 == 1:
                        yield
                if with_bias:
                    nc.tensor.matmul(
                        pv[:, n0:n0 + nw],
                        ones[0:1, 0:128],
                        _r(brow["bv"][0:1, n0:n0 + nw]),
                        start=False,
                        stop=True,
                    )
            nc.vector.tensor_copy(
                Vaug[:, sc, :, 0:DH],
                pv[:, 0:D].rearrange("p (h e) -> p h e", h=H),
            )
            yield

        def s_chunk(hp, take=2, must=None):
            """Scores + exp for head pair hp; returns the e tile.

            `must` is the generator producing this head pair's QT/KT — it is
            drained first so every score matmul reads completed copies
            (emission order defines the dataflow graph)."""
            if must is not None:
                for _ in must:
                    pass
            e = ep.tile([128, SC, 1024], BF, tag="e", name=f"e{hp}")
            for kc in range(SC):
                st = ssp.tile([128, 1024], FP, tag="ss", name=f"st{hp}_{kc}")
                for j in range(2):
                    off = j * DH
                    nc.tensor.matmul(
                        st[:, j * 512:j * 512 + 512],
                        KT[off:off + DH, hp, kc * 128:(kc + 1) * 128],
                        QT[off:off + DH, hp, :],
                        start=True,
                        stop=True,
                        tile_position=(off, 0),
                    )
                nc.scalar.activation(e[:, kc, :], st[:], AF.Exp, scale=0.125)
                take_filler(take)
            return e

        def tq_part(hp):
            """PE-transpose ctxC[hp] -> TQ[hp] (the xbar DMA transpose's
            completion semaphore is unreliable for PE consumers in real
            execution, so stay on the well-trodden matmul path)."""
            ptq = pjp.tile([128, 1024], FP, tag="pj", name=f"ptq{hp}")
            for qc in range(4):
                nc.tensor.transpose(
                    _r(ptq[:, qc * 128:(qc + 1) * 128]),
                    ctxC[:, hp, qc * 128:(qc + 1) * 128],
                    identR[:],
                )
            nc.vector.tensor_copy(
                TQ[hp][:].rearrange("p a b -> p (a b)"), _r(ptq[:, 0:512])
            )

        def ctx_gen(hp, e):
            """ctx_norm as filler quanta (for head pairs that only gate a
            LATER s_chunk's e-buffer, not the one they interleave into)."""
            rec = smallp.tile([128, 2, 4], FP, tag="rec", name=f"rec{hp}")
            for j, eng in ((0, nc.vector), (1, nc.vector)):
                pc = cxp.tile([128, 4, DH + 1], FP, tag=f"cx{j}", name=f"cx{hp}_{j}")
                h = 2 * hp + j
                for qc in range(4):
                    for kc in range(SC):
                        nc.tensor.matmul(
                            pc[:, qc, :],
                            e[:, kc, j * 512 + qc * 128:j * 512 + qc * 128 + 128],
                            Vaug[:, kc, h, :],
                            start=(kc == 0),
                            stop=(kc == SC - 1),
                        )
                    if qc == 1:
                        yield
                nc.vector.reciprocal(rec[:, j, :], pc[:, :, DH:DH + 1])
                for qc in range(4):
                    eng.tensor_scalar_mul(
                        ctxC[:, hp, qc * 128 + j * DH:qc * 128 + (j + 1) * DH],
                        pc[:, qc, 0:DH],
                        rec[:, j, qc:qc + 1],
                    )
                yield

        def ctx_norm(hp, e):
            """ALT-layout ctx + normalization for head pair hp."""
            rec = smallp.tile([128, 2, 4], FP, tag="rec", name=f"rec{hp}")
            for j, eng in ((0, nc.vector), (1, nc.vector)):
                pc = cxp.tile([128, 4, DH + 1], FP, tag=f"cx{j}", name=f"cx{hp}_{j}")
                h = 2 * hp + j
                for qc in range(4):
                    for kc in range(SC):
                        nc.tensor.matmul(
                            pc[:, qc, :],
                            e[:, kc, j * 512 + qc * 128:j * 512 + qc * 128 + 128],
                            Vaug[:, kc, h, :],
                            start=(kc == 0),
                            stop=(kc == SC - 1),
                        )
                nc.vector.reciprocal(rec[:, j, :], pc[:, :, DH:DH + 1])
                for qc in range(4):
                    # ctxC col layout qc*128 + d_local: the xbar transpose's
                    # 3D out [128, 4, 128] maps out[p, f1, f2] = in[f2, f1*128+p]
                    eng.tensor_scalar_mul(
                        ctxC[:, hp, qc * 128 + j * DH:qc * 128 + (j + 1) * DH],
                        pc[:, qc, 0:DH],
                        rec[:, j, qc:qc + 1],
                    )

        def meanv_gen():
            """Masked-row tail: mean_k(V) @ Wo + broadcast rows 512:1024."""
            pm = pjp.tile([128, 1024], FP, tag="pj", name="pm")
            for c in range(DC):
                for k in range(DC):
                    nc.tensor.matmul(
                        pm[:, c:c + 1],
                        wv[:, k, c * 128:(c + 1) * 128].bitcast(FP),
                        xsum[:, k:k + 1].bitcast(FP),

                        start=(k == 0),
                        stop=(k == DC - 1),
                    )
                if c % 2 == 1:
                    yield
            if with_bias:
                nc.vector.scalar_tensor_tensor(
                    mvt[:], pm[:, 0:DC], 1.0 / S,
                    bvT[:], op0=mybir.AluOpType.mult, op1=mybir.AluOpType.add,
                )
            else:
                nc.vector.tensor_scalar_mul(mvt[:], pm[:, 0:DC], 1.0 / S)
            yield
            pt2 = pjp.tile([128, 1024], FP, tag="pj", name="pt2")
            for n0, nw in NT:
                for k in range(DC):
                    nc.tensor.matmul(
                        pt2[0:1, n0:n0 + nw],
                        mvt[:, k:k + 1],
                        wob[:, k, n0:n0 + nw],
                        start=(k == 0),
                        stop=(not with_bias and k == DC - 1),
                    )
                    if k % 2 == 1:
                        yield
                if with_bias:
                    nc.tensor.matmul(
                        pt2[0:1, n0:n0 + nw],
                        onesB[0:1, 0:1],
                        browB[0:1, n0:n0 + nw],
                        start=False,
                        stop=True,
                    )
            trow = constp.tile([1, D], FP, tag="trow")
            nc.vector.tensor_copy(trow[:], pt2[0:1, 0:D])
            nc.gpsimd.partition_broadcast(ttile[:], trow[0:1, :])
            for sc in range(SH // 128, SC):
                nc.sync.dma_start(
                    out=out[sc * 128:(sc + 1) * 128, :], in_=ttile[:]
                )
            yield

        def ctx_old(hp, e):
            """OLD-layout ctx for the last head pair: ctxT5 [128 d, 512 q].
            Both heads share one PSUM tile (regions 0:512 / 512:1024)."""
            p5 = pjp.tile([128, 1024], FP, tag="pj", name="p5")
            for j in range(2):
                h = 2 * hp + j
                for kc in range(SC):
                    nc.tensor.matmul(
                        p5[0:DH + 1, j * 512:j * 512 + 512],
                        Vaug[:, kc, h, :],
                        e[:, kc, j * 512:j * 512 + 512],
                        start=(kc == 0),
                        stop=(kc == SC - 1),
                    )
            for j in range(2):
                recrow = smallp.tile([1, SH], FP, tag="recrow", name=f"rr{j}")
                nc.vector.reciprocal(recrow[:], p5[DH:DH + 1, j * 512:j * 512 + 512])
                bsb = smallp.tile([DH, SH], FP, tag="bsb", name=f"bsb{j}")
                nc.gpsimd.partition_broadcast(bsb[:], recrow[0:1, :])
                nc.vector.tensor_mul(
                    ctxT5[j * DH:(j + 1) * DH, :],
                    p5[0:DH, j * 512:j * 512 + 512],
                    bsb[:],
                )

        # ---------------- main schedule ----------------
        # drain one generator fully
        def run_all(g):
            for _ in g:
                pass

        run_all(qkt_gen(0))
        g = [qkt_gen(c) for c in range(1, DC)]
        fillers.append(g[0])
        es = [s_chunk(0, take=4)]
        fillers.append(g[1])
        fillers.append(v_gen(0))
        fillers.append(v_gen(1))
        es.append(s_chunk(1, take=4, must=g[0]))
        fillers.append(v_gen(2))
        fillers.append(v_gen(3))
        fillers.append(v_gen(4))
        fillers.append(g[2])
        es.append(s_chunk(2, take=4, must=g[1]))
        fillers.append(v_gen(5))
        fillers.append(v_gen(6))
        fillers.append(v_gen(7))
        fillers.append(g[3])
        es.append(s_chunk(3, take=4, must=g[2]))
        drain_fillers()
        ctx_norm(0, es[0])
        fillers.append(g[4])
        fillers.append(ctx_gen(1, es[1]))
        es.append(s_chunk(4, take=4, must=g[3]))
        drain_fillers()
        # column sums of x for the masked-row tail (after the qkt5 copies so
        # it does not block them on the in-order DVE queue)
        for c in range(DC):
            nc.vector.tensor_reduce(
                xsum[:, c:c + 1, None], xT[:, c, :].bitcast(FP),
                axis=mybir.AxisListType.X, op=mybir.AluOpType.add,
            )
        tq_part(0)
        fillers.append(meanv_gen())
        es.append(s_chunk(5, take=4, must=g[4]))
        drain_fillers()
        ctx_norm(2, es[2])
        tq_part(1)
        ctx_norm(3, es[3])
        tq_part(2)
        ctx_norm(4, es[4])
        tq_part(3)

        tq_part(4)
        # last head pair in the OLD layout (no transpose on the tail)
        ctx_old(ALT_HP, es[ALT_HP])

        # ---------------- output projection, rows 0:512 ----------------
        # qc 0/1: accumulate the TQ blocks (dc 0..4) before the exp-gated
        # ctx_old, close with the ctxT5 block after; qc 2/3 run whole
        for qc in range(4):
            po = ssp.tile([128, 1024], FP, tag="ss", name=f"po{qc}")
            osb = op_.tile([128, D], FP, tag="osb", name=f"osb{qc}")
            for n0, nw in NT:
                for dc in range(ALT_HP):
                    nc.tensor.matmul(
                        po[:, n0:n0 + nw],
                        TQ[dc][:, qc, :],
                        wob[:, dc, n0:n0 + nw],
                        start=(dc == 0),
                        stop=False,
                    )
                nc.tensor.matmul(
                    po[:, n0:n0 + nw],
                    ctxT5[:, qc * 128:(qc + 1) * 128],
                    wob[:, ALT_HP, n0:n0 + nw],
                    start=False,
                    stop=(not with_bias),
                )
                if with_bias:
                    nc.tensor.matmul(
                        po[:, n0:n0 + nw],
                        onesB[0:1, 0:128],
                        browB[0:1, n0:n0 + nw],
                        start=False,
                        stop=True,
                    )
                nc.vector.tensor_copy(osb[:, n0:n0 + nw], po[:, n0:n0 + nw])
                nc.sync.dma_start(
                    out=out[qc * 128:(qc + 1) * 128, n0:n0 + nw],
                    in_=osb[:, n0:n0 + nw],
                )

def build_nc(with_bias=True):
    nc = bacc.Bacc("TRN2", target_bir_lowering=False, debug=False, num_devices=NCORES)
    x = nc.dram_tensor("x", [S, D], FP, kind="ExternalInput").ap()
    W = {
        nm: nc.dram_tensor(nm, [D, D], FP, kind="ExternalInput").ap()
        for nm in ("Wq", "Wk", "Wv", "Wo")
    }
    bvec = {
        nm: nc.dram_tensor(nm, [D], FP, kind="ExternalInput").ap()
        for nm in ("bq", "bk", "bv", "bo")
    }
    out = nc.dram_tensor("out", [S, D], FP, kind="ExternalOutput").ap()
    with tile.TileContext(nc) as tc:
        _body(tc, out, x, W, bvec, with_bias=with_bias)
    nc.compile()
    return nc


def kernel(hidden_states, Wq, bq, Wk, bk, Wv, bv, Wo, bo, _trace=False):
    hidden_states = np.ascontiguousarray(np.asarray(hidden_states, dtype=np.float32))
    shared = {
        "Wq": np.ascontiguousarray(np.asarray(Wq, np.float32)),
        "Wk": np.ascontiguousarray(np.asarray(Wk, np.float32)),
        "Wv": np.ascontiguousarray(np.asarray(Wv, np.float32)),
        "Wo": np.ascontiguousarray(np.asarray(Wo, np.float32)),
        "bq": np.ascontiguousarray(np.asarray(bq, np.float32)),
        "bk": np.ascontiguousarray(np.asarray(bk, np.float32)),
        "bv": np.ascontiguousarray(np.asarray(bv, np.float32)),
        "bo": np.ascontiguousarray(np.asarray(bo, np.float32)),
    }
    with_bias = any(np.any(shared[b]) for b in ("bq", "bk", "bv", "bo"))
    nc = build_nc(with_bias=with_bias)
    in_maps = [{"x": hidden_states[i], **shared} for i in range(NCORES)]
    res = run_bass_kernel_spmd(
        nc, in_maps, core_ids=list(range(NCORES)), trace=_trace
    )
    out = np.stack([res.results[i]["out"] for i in range(NCORES)], axis=0)
    if _trace:
        kernel.last_results = res
    return out


if __name__ == "__main__":
    rng = np.random.default_rng(0)
    ins = {
        "hidden_states": rng.standard_normal((B, S, D), dtype=np.float32),
        **{w: (rng.standard_normal((D, D)) / np.sqrt(D)).astype(np.float32) for w in ("Wq", "Wk", "Wv", "Wo")},
        **{b: np.zeros(D, np.float32) for b in ("bq", "bk", "bv", "bo")},
    }
    o = kernel(**ins)
    print("kernel ran, out shape", o.shape)


# revision 66
# speedup vs baseline: 1.0060x; 1.0015x over previous
"""Trainium2 Bass kernel for nn_Attention_28020366639391 (sparse attention).

Math (per batch element b, reference semantics):
    q/k/v = x @ W{q,k,v} + b{q,k,v}, split into 12 heads of 64
    scores = q k^T / 8 ; rows >= 512 zeroed pre-softmax
    -> rows >= 512 have uniform probs = 1/1024 -> ctx row = mean_k(v)
    out = concat_heads(ctx) @ Wo + bo

Sharding: data-parallel on batch. 8 batch elements -> 8 NeuronCores, no
collectives. Each core gets x=[1024,768] + the full weights and computes
out=[1024,768].

Per-core dataflow (v2 — engine-balanced around the PE roofline):
    xT   [768,1024] fp32r = PE-transpose of x (d' on partitions)
    QT/KT bf16 = W{q,k}^T @ xT (Wq/Wk streamed as [768,128] column blocks,
                 2 blocks live at a time)
    V    bf16 per 128-row chunk -> Vaug [k, 12h, 64+ones] (ones column gives
                 softmax row sums for free in the ctx matmul)
    per head pair hp, key chunk kc: scores sT [kc 128, 1024] PSUM tile
                 (head 2hp in cols 0:512, head 2hp+1 in 512:1024),
                 one exp -> e[hp] bf16 [128, kc, 1024] on ACT
    ctx (heads 0-9, "ALT" layout): out [q 128, 65] bf16 matmuls
                 lhsT = e slice [k,128q], rhs = Vaug head slice [k,65];
                 col 64 = softmax denominator. Normalize via per-partition
                 reciprocal + tensor_scalar on DVE -> ctxC fp32r, then PE
                 transposes (lagged one head pair) -> TQ bf16 (d on parts).
    ctx (heads 10-11, "OLD" layout): out [65, 512] keeps d on partitions so
                 the out-projection's last contraction block needs no
                 transpose (shortens the critical tail).
    out rows 0:512   = TQ/ctxT5 blocks @ Wo(bf16)
    out rows 512:1024 = broadcast of (mean_k V) @ Wo (from column sums of x)

All projections run in fp32r (1 cycle/row on the PE); attention ctx runs in
bf16 (needed for 1 cycle/row at free-size 65). Weight tensors stay fp32r in
SBUF except Wo, which is copied to bf16 on GpSimd to match the bf16 ctx
operands (the PE cannot mix 32-bit and 16-bit operands).
"""

import os

import numpy as np

import concourse.bass as bass
import concourse.mybir as mybir
import concourse.tile as tile
from concourse import bacc
from concourse.bass_utils import run_bass_kernel_spmd
from concourse.masks import make_identity

B, S, D, H, DH = 8, 1024, 768, 12, 64
SH = 512            # active (unmasked) query rows = patches//2
DC = D // 128       # 6 chunks of the model dim
SC = S // 128       # 8 chunks of the sequence dim
HP = H // 2         # 6 head pairs
NCORES = 8
FP = mybir.dt.float32
FPR = mybir.dt.float32r
BF = mybir.dt.bfloat16
AF = mybir.ActivationFunctionType
NT = ((0, 512), (512, 256))  # free-dim tiling of a 768-wide output
ALT_HP = HP - 1     # head pairs 0..4 use the ALT (q-partition) ctx layout


def _r(ap):
    """View an fp32 AP as float32r so the PE runs at full rate."""
    return ap.bitcast(FPR)


def _body(tc, out, x, W, bvec, with_bias=True):
    nc = tc.nc
    from contextlib import ExitStack

    with ExitStack() as ctx:
        ctx.enter_context(
            nc.allow_low_precision(reason="fp32r/bf16 PE paths by design")
        )
        constp = ctx.enter_context(tc.tile_pool(name="const", bufs=1))
        wvp = ctx.enter_context(tc.tile_pool(name="wv", bufs=1))
        wop = ctx.enter_context(tc.tile_pool(name="wo", bufs=1))
        wqkp = ctx.enter_context(tc.tile_pool(name="wqk", bufs=2))
        xtp = ctx.enter_context(tc.tile_pool(name="xt", bufs=1))
        qkp = ctx.enter_context(tc.tile_pool(name="qk", bufs=1))
        vap = ctx.enter_context(tc.tile_pool(name="va", bufs=1))
        ep = ctx.enter_context(tc.tile_pool(name="e", bufs=4))
        cxc = ctx.enter_context(tc.tile_pool(name="cxc", bufs=1))
        smallp = ctx.enter_context(tc.tile_pool(name="small", bufs=1))
        op_ = ctx.enter_context(tc.tile_pool(name="o", bufs=2))
        xnp = ctx.enter_context(tc.tile_pool(name="xn", bufs=1))
        # PSUM: scores 2x[128,1024] (4 banks) + projections 1x (2 banks) +
        # two ctx accumulators (1 bank each) = 8 banks exactly
        ssp = ctx.enter_context(tc.tile_pool(name="ss", bufs=2, space="PSUM"))
        pjp = ctx.enter_context(tc.tile_pool(name="pj", bufs=1, space="PSUM"))
        cxp = ctx.enter_context(tc.tile_pool(name="cx", bufs=1, space="PSUM"))

        # ---------------- constants ----------------
        ident = constp.tile([128, 128], FP, tag="ident")
        make_identity(nc, ident[:])
        identR = constp.tile([128, 128], FPR, tag="identR")
        nc.vector.tensor_copy(identR[:], ident[:])
        onesF = constp.tile([1, 512], FP, tag="onesF")
        nc.vector.memset(onesF[:], 1.0)
        ones = constp.tile([1, 512], FPR, tag="ones")
        nc.vector.tensor_copy(ones[:], onesF[:])
        onesB = constp.tile([1, 512], BF, tag="onesB")
        nc.vector.tensor_copy(onesB[:], onesF[:])

        # ---------------- input DMAs, in schedule order ----------------
        # x first (gates everything), then Wq/Wk column blocks just-in-time
        # per head pair, Wv row blocks interleaved, Wo last.
        # x in 4 DMA instructions of 2 row-chunks each (HWDGE descriptor
        # generation serializes at ~625ns/instruction — batch aggressively)
        xn = []
        for g, (r0, nr) in enumerate(((0, 2), (2, 2), (4, 2), (6, 2))):
            t = xnp.tile([128, 2, D], FPR, tag=f"xh{g % 2}", name=f"xn{g}")
            nc.sync.dma_start(
                out=t[:, 0:nr, :],
                in_=_r(x[r0 * 128:(r0 + nr) * 128, :]).rearrange(
                    "(s p) d -> p s d", p=128
                ),
            )
            xn.append((t, r0, nr))

        wq = []
        wk = []

        def load_wqk(c):
            for nm, lst in (("Wq", wq), ("Wk", wk)):
                t = wqkp.tile([128, DC, 128], FPR, tag=nm, name=f"{nm}{c}")
                nc.sync.dma_start(
                    out=t[:],
                    in_=_r(W[nm][:, c * 128:(c + 1) * 128]).rearrange(
                        "(k p) c -> p k c", p=128
                    ),
                )
                lst.append(t)

        wv = wvp.tile([128, DC, D], FPR, tag="Wv", name="w_Wv")

        load_wqk(0)
        load_wqk(1)
        nc.sync.dma_start(
            out=wv[:], in_=_r(W["Wv"][:, :]).rearrange("(k p) d -> p k d", p=128)
        )
        load_wqk(2)
        load_wqk(3)
        load_wqk(4)
        load_wqk(5)

        # Wo: stage fp32 in the x slots, convert to bf16 on GpSimd
        wob = wop.tile([128, DC, D], BF, tag="Wob", name="w_Wob")
        for g in range(3):
            wos = xnp.tile([128, 2, D], FPR, tag=f"xh{g % 2}", name=f"wos{g}")
            nc.sync.dma_start(
                out=wos[:],
                in_=_r(W["Wo"][g * 256:(g + 1) * 256, :]).rearrange(
                    "(k p) d -> p k d", p=128
                ),
            )
            nc.gpsimd.tensor_copy(wob[:, 2 * g:2 * g + 2, :], wos[:].bitcast(FP))

        brow = {}
        if with_bias:
            for nm in ("bq", "bk", "bv", "bo"):
                t = constp.tile([1, D], FPR, tag=f"brow_{nm}", name=f"brow_{nm}")
                nc.sync.dma_start(out=t[:], in_=_r(bvec[nm][None, :]))
                brow[nm] = t
            bvT = constp.tile([128, DC], FP, tag="bvT")
            for c in range(DC):
                nc.sync.dma_start(
                    out=bvT[:, c:c + 1], in_=bvec["bv"][c * 128:(c + 1) * 128, None]
                )
            browB = constp.tile([1, D], BF, tag="browB")
            nc.vector.tensor_copy(browB[:], brow["bo"][:].bitcast(FP))

        # ---------------- persistent tensors ----------------
        xT = xtp.tile([128, DC, S], FPR, tag="xT")
        QT = qkp.tile([128, DC, SH], BF, tag="QT")
        KT = qkp.tile([128, DC, S], BF, tag="KT")
        Vaug = vap.tile([128, SC, H, DH + 1], BF, tag="Vaug")
        vones = constp.tile([128, SC * H], FP, tag="vones")
        nc.vector.memset(vones[:], 1.0)
        nc.vector.tensor_copy(
            Vaug[:, :, :, DH:DH + 1],
            vones[:].rearrange("p (k h) -> p k h", k=SC)[:, :, :, None],
        )
        # ctxC: per head pair, 512 columns interleaved as (d_local, qc) ->
        # d_local*4 + qc, so ONE xbar transpose per head pair yields
        # TQ[hp][p, qc, q] = ctxT[d = hp*128 + p, qc*128 + q]
        ctxC = cxc.tile([128, ALT_HP, 512], FPR, tag="ctxC")
        TQ = [cxc.tile([128, 4, 128], BF, tag=f"TQ{hp}", name=f"TQ{hp}")
              for hp in range(ALT_HP)]
        ctxT5 = cxc.tile([128, SH], BF, tag="ctxT5")
        xsum = constp.tile([128, DC], FPR, tag="xsum")
        mvt = constp.tile([128, DC], BF, tag="mvt")
        ttile = constp.tile([128, D], FP, tag="ttile")

        # ---------------- x transposes (PE, fp32r: 1.5 cycles/row) --------
        for t, r0, nr in xn:
            for s in range(nr):
                sc = r0 + s
                pt = ssp.tile([128, 1024], FP, tag="ss", name=f"ptx{sc}")
                for c in range(DC):
                    nc.tensor.transpose(
                        _r(pt[:, c * 128:(c + 1) * 128]),
                        t[:, s, c * 128:(c + 1) * 128],
                        identR[:],
                    )
                ptv = pt[:, 0:D].rearrange("p (c s) -> p c s", c=DC)
                nc.vector.tensor_copy(xT[:, :, sc * 128:(sc + 1) * 128], _r(ptv))

        # ------------- quantum generators (PE filler interleaving) -------
        # The exp stream on ACT (~1038ns per [128,1024] tile) is 2.4x slower
        # than the two score matmuls feeding it (~426ns). s_chunk emits
        # filler quanta of independent PE work between score pairs so the PE
        # never throttles to the ACT rate.
        from collections import deque

        fillers = deque()

        def take_filler(n):
            done = 0
            while done < n and fillers:
                try:
                    next(fillers[0])
                    done += 1
                except StopIteration:
                    fillers.popleft()

        def drain_fillers():
            while fillers:
                try:
                    next(fillers[0])
                except StopIteration:
                    fillers.popleft()

        def qkt_gen(c):
            """QT/KT projection chunk c in ~2-matmul quanta."""
            pq = pjp.tile([128, 1024], FP, tag="pj", name=f"pq{c}")
            for k in range(DC):
                nc.tensor.matmul(
                    pq[:, 0:SH],
                    wq[c][:, k, :],
                    xT[:, k, 0:SH],
                    start=(k == 0),
                    stop=(not with_bias and k == DC - 1),
                )
                if k % 2 == 1:
                    yield
            if with_bias:
                nc.tensor.matmul(
                    pq[:, 0:SH],
                    _r(brow["bq"][0:1, c * 128:(c + 1) * 128]),
                    ones[0:1, 0:SH],
                    start=False,
                    stop=True,
                )
            nc.vector.tensor_copy(QT[:, c, :], pq[:, 0:SH])
            yield
            pk = pjp.tile([128, 1024], FP, tag="pj", name=f"pk{c}")
            for sg in range(2):
                for k in range(DC):
                    nc.tensor.matmul(
                        pk[:, sg * 512:sg * 512 + 512],
                        wk[c][:, k, :],
                        xT[:, k, sg * 512:sg * 512 + 512],
                        start=(k == 0),
                        stop=(not with_bias and k == DC - 1),
                    )
                    if k % 2 == 1:
                        yield
                if with_bias:
                    nc.tensor.matmul(
                        pk[:, sg * 512:sg * 512 + 512],
                        _r(brow["bk"][0:1, c * 128:(c + 1) * 128]),
                        ones[0:1, 0:512],
                        start=False,
                        stop=True,
                    )
                nc.vector.tensor_copy(
                    KT[:, c, sg * 512:sg * 512 + 512],
                    pk[:, sg * 512:sg * 512 + 512],
                )
                yield

        def v_gen(sc):
            """V projection chunk sc in ~2-matmul quanta."""
            pv = pjp.tile([128, 1024], FP, tag="pj", name=f"pv{sc}")
            for n0, nw in NT:
                for k in range(DC):
                    nc.tensor.matmul(
                        pv[:, n0:n0 + nw],
                        xT[:, k, sc * 128:(sc + 1) * 128],
                        wv[:, k, n0:n0 + nw],
                        start=(k == 0),
                        stop=(not with_bias and k == DC - 1),
                    )
                    if k % 2 == 1:
                        yield
                if with_bias:
                    nc.tensor.matmul(
                        pv[:, n0:n0 + nw],
                        ones[0:1, 0:128],
                        _r(brow["bv"][0:1, n0:n0 + nw]),
                        start=False,
                        stop=True,
                    )
            nc.vector.tensor_copy(
                Vaug[:, sc, :, 0:DH],
                pv[:, 0:D].rearrange("p (h e) -> p h e", h=H),
            )
            yield

        def s_chunk(hp, take=2, must=None):
            """Scores + exp for head pair hp; returns the e tile.

            `must` is the generator producing this head pair's QT/KT — it is
            drained first so every score matmul reads completed copies
            (emission order defines the dataflow graph)."""
            if must is not None:
                for _ in must:
                    pass
            e = ep.tile([128, SC, 1024], BF, tag="e", name=f"e{hp}")
            for kc in range(SC):
                st = ssp.tile([128, 1024], FP, tag="ss", name=f"st{hp}_{kc}")
                for j in range(2):
                    off = j * DH
                    nc.tensor.matmul(
                        st[:, j * 512:j * 512 + 512],
                        KT[off:off + DH, hp, kc * 128:(kc + 1) * 128],
                        QT[off:off + DH, hp, :],
                        start=True,
                        stop=True,
                        tile_position=(off, 0),
                    )
                nc.scalar.activation(e[:, kc, :], st[:], AF.Exp, scale=0.125)
                take_filler(take)
            return e

        def tq_part(hp):
            """PE-transpose ctxC[hp] -> TQ[hp] (the xbar DMA transpose's
            completion semaphore is unreliable for PE consumers in real
            execution, so stay on the well-trodden matmul path)."""
            ptq = pjp.tile([128, 1024], FP, tag="pj", name=f"ptq{hp}")
            for qc in range(4):
                nc.tensor.transpose(
                    _r(ptq[:, qc * 128:(qc + 1) * 128]),
                    ctxC[:, hp, qc * 128:(qc + 1) * 128],
                    identR[:],
                )
            nc.vector.tensor_copy(
                TQ[hp][:].rearrange("p a b -> p (a b)"), _r(ptq[:, 0:512])
            )

        def tq_gen(hp):
            tq_part(hp)
            yield

        def ctx_gen(hp, e):
            """ctx_norm as filler quanta (for head pairs that only gate a
            LATER s_chunk's e-buffer, not the one they interleave into)."""
            rec = smallp.tile([128, 2, 4], FP, tag="rec", name=f"rec{hp}")
            for j, eng in ((0, nc.vector), (1, nc.vector)):
                pc = cxp.tile([128, 4, DH + 1], FP, tag=f"cx{j}", name=f"cx{hp}_{j}")
                h = 2 * hp + j
                for qc in range(4):
                    for kc in range(SC):
                        nc.tensor.matmul(
                            pc[:, qc, :],
                            e[:, kc, j * 512 + qc * 128:j * 512 + qc * 128 + 128],
                            Vaug[:, kc, h, :],
                            start=(kc == 0),
                            stop=(kc == SC - 1),
                        )
                    if qc == 1:
                        yield
                nc.vector.reciprocal(rec[:, j, :], pc[:, :, DH:DH + 1])
                for qc in range(4):
                    eng.tensor_scalar_mul(
                        ctxC[:, hp, qc * 128 + j * DH:qc * 128 + (j + 1) * DH],
                        pc[:, qc, 0:DH],
                        rec[:, j, qc:qc + 1],
                    )
                yield

        def ctx_norm(hp, e):
            """ALT-layout ctx + normalization for head pair hp."""
            rec = smallp.tile([128, 2, 4], FP, tag="rec", name=f"rec{hp}")
            for j, eng in ((0, nc.vector), (1, nc.vector)):
                pc = cxp.tile([128, 4, DH + 1], FP, tag=f"cx{j}", name=f"cx{hp}_{j}")
                h = 2 * hp + j
                for qc in range(4):
                    for kc in range(SC):
                        nc.tensor.matmul(
                            pc[:, qc, :],
                            e[:, kc, j * 512 + qc * 128:j * 512 + qc * 128 + 128],
                            Vaug[:, kc, h, :],
                            start=(kc == 0),
                            stop=(kc == SC - 1),
                        )
                nc.vector.reciprocal(rec[:, j, :], pc[:, :, DH:DH + 1])
                for qc in range(4):
                    # ctxC col layout qc*128 + d_local: the xbar transpose's
                    # 3D out [128, 4, 128] maps out[p, f1, f2] = in[f2, f1*128+p]
                    eng.tensor_scalar_mul(
                        ctxC[:, hp, qc * 128 + j * DH:qc * 128 + (j + 1) * DH],
                        pc[:, qc, 0:DH],
                        rec[:, j, qc:qc + 1],
                    )

        def meanv_gen():
            """Masked-row tail: mean_k(V) @ Wo + broadcast rows 512:1024."""
            pm = pjp.tile([128, 1024], FP, tag="pj", name="pm")
            for c in range(DC):
                for k in range(DC):
                    nc.tensor.matmul(
                        pm[:, c:c + 1],
                        wv[:, k, c * 128:(c + 1) * 128].bitcast(FP),
                        xsum[:, k:k + 1].bitcast(FP),

                        start=(k == 0),
                        stop=(k == DC - 1),
                    )
                if c % 2 == 1:
                    yield
            if with_bias:
                nc.vector.scalar_tensor_tensor(
                    mvt[:], pm[:, 0:DC], 1.0 / S,
                    bvT[:], op0=mybir.AluOpType.mult, op1=mybir.AluOpType.add,
                )
            else:
                nc.vector.tensor_scalar_mul(mvt[:], pm[:, 0:DC], 1.0 / S)
            yield
            pt2 = pjp.tile([128, 1024], FP, tag="pj", name="pt2")
            for n0, nw in NT:
                for k in range(DC):
                    nc.tensor.matmul(
                        pt2[0:1, n0:n0 + nw],
                        mvt[:, k:k + 1],
                        wob[:, k, n0:n0 + nw],
                        start=(k == 0),
                        stop=(not with_bias and k == DC - 1),
                    )
                    if k % 2 == 1:
                        yield
                if with_bias:
                    nc.tensor.matmul(
                        pt2[0:1, n0:n0 + nw],
                        onesB[0:1, 0:1],
                        browB[0:1, n0:n0 + nw],
                        start=False,
                        stop=True,
                    )
            trow = constp.tile([1, D], FP, tag="trow")
            nc.vector.tensor_copy(trow[:], pt2[0:1, 0:D])
            nc.gpsimd.partition_broadcast(ttile[:], trow[0:1, :])
            for sc in range(SH // 128, SC):
                nc.sync.dma_start(
                    out=out[sc * 128:(sc + 1) * 128, :], in_=ttile[:]
                )
            yield

        def ctx_old(hp, e):
            """OLD-layout ctx for the last head pair: ctxT5 [128 d, 512 q].
            Both heads share one PSUM tile (regions 0:512 / 512:1024)."""
            p5 = pjp.tile([128, 1024], FP, tag="pj", name="p5")
            for j in range(2):
                h = 2 * hp + j
                for kc in range(SC):
                    nc.tensor.matmul(
                        p5[0:DH + 1, j * 512:j * 512 + 512],
                        Vaug[:, kc, h, :],
                        e[:, kc, j * 512:j * 512 + 512],
                        start=(kc == 0),
                        stop=(kc == SC - 1),
                    )
            for j in range(2):
                recrow = smallp.tile([1, SH], FP, tag="recrow", name=f"rr{j}")
                nc.vector.reciprocal(recrow[:], p5[DH:DH + 1, j * 512:j * 512 + 512])
                bsb = smallp.tile([DH, SH], FP, tag="bsb", name=f"bsb{j}")
                nc.gpsimd.partition_broadcast(bsb[:], recrow[0:1, :])
                nc.vector.tensor_mul(
                    ctxT5[j * DH:(j + 1) * DH, :],
                    p5[0:DH, j * 512:j * 512 + 512],
                    bsb[:],
                )

        # ---------------- main schedule ----------------
        # drain one generator fully
        def run_all(g):
            for _ in g:
                pass

        run_all(qkt_gen(0))
        g = [qkt_gen(c) for c in range(1, DC)]
        fillers.append(g[0])
        es = [s_chunk(0, take=4)]
        fillers.append(g[1])
        fillers.append(v_gen(0))
        fillers.append(v_gen(1))
        es.append(s_chunk(1, take=4, must=g[0]))
        fillers.append(v_gen(2))
        fillers.append(v_gen(3))
        fillers.append(v_gen(4))
        fillers.append(g[2])
        es.append(s_chunk(2, take=4, must=g[1]))
        fillers.append(v_gen(5))
        fillers.append(v_gen(6))
        fillers.append(v_gen(7))
        fillers.append(g[3])
        fillers.append(ctx_gen(0, es[0]))
        es.append(s_chunk(3, take=4, must=g[2]))
        drain_fillers()
        fillers.append(g[4])
        fillers.append(ctx_gen(1, es[1]))
        fillers.append(tq_gen(0))
        es.append(s_chunk(4, take=4, must=g[3]))
        drain_fillers()
        # column sums of x for the masked-row tail (after the qkt5 copies so
        # it does not block them on the in-order DVE queue)
        for c in range(DC):
            nc.vector.tensor_reduce(
                xsum[:, c:c + 1, None], xT[:, c, :].bitcast(FP),
                axis=mybir.AxisListType.X, op=mybir.AluOpType.add,
            )
        fillers.append(meanv_gen())
        es.append(s_chunk(5, take=4, must=g[4]))
        drain_fillers()
        ctx_norm(2, es[2])
        tq_part(1)
        ctx_norm(3, es[3])
        tq_part(2)
        ctx_norm(4, es[4])
        tq_part(3)

        tq_part(4)
        # last head pair in the OLD layout (no transpose on the tail)
        ctx_old(ALT_HP, es[ALT_HP])

        # ---------------- output projection, rows 0:512 ----------------
        # qc 0/1: accumulate the TQ blocks (dc 0..4) before the exp-gated
        # ctx_old, close with the ctxT5 block after; qc 2/3 run whole
        for qc in range(4):
            po = ssp.tile([128, 1024], FP, tag="ss", name=f"po{qc}")
            osb = op_.tile([128, D], FP, tag="osb", name=f"osb{qc}")
            for n0, nw in NT:
                for dc in range(ALT_HP):
                    nc.tensor.matmul(
                        po[:, n0:n0 + nw],
                        TQ[dc][:, qc, :],
                        wob[:, dc, n0:n0 + nw],
                        start=(dc == 0),
                        stop=False,
                    )
                nc.tensor.matmul(
                    po[:, n0:n0 + nw],
                    ctxT5[:, qc * 128:(qc + 1) * 128],
                    wob[:, ALT_HP, n0:n0 + nw],
                    start=False,
                    stop=(not with_bias),
                )
                if with_bias:
                    nc.tensor.matmul(
                        po[:, n0:n0 + nw],
                        onesB[0:1, 0:128],
                        browB[0:1, n0:n0 + nw],
                        start=False,
                        stop=True,
                    )
                nc.vector.tensor_copy(osb[:, n0:n0 + nw], po[:, n0:n0 + nw])
                nc.sync.dma_start(
                    out=out[qc * 128:(qc + 1) * 128, n0:n0 + nw],
                    in_=osb[:, n0:n0 + nw],
                )

def build_nc(with_bias=True):
    nc = bacc.Bacc("TRN2", target_bir_lowering=False, debug=False, num_devices=NCORES)
    x = nc.dram_tensor("x", [S, D], FP, kind="ExternalInput").ap()
    W = {
        nm: nc.dram_tensor(nm, [D, D], FP, kind="ExternalInput").ap()
        for nm in ("Wq", "Wk", "Wv", "Wo")
    }
    bvec = {
        nm: nc.dram_tensor(nm, [D], FP, kind="ExternalInput").ap()
        for nm in ("bq", "bk", "bv", "bo")
    }
    out = nc.dram_tensor("out", [S, D], FP, kind="ExternalOutput").ap()
    with tile.TileContext(nc) as tc:
        _body(tc, out, x, W, bvec, with_bias=with_bias)
    nc.compile()
    return nc


def kernel(hidden_states, Wq, bq, Wk, bk, Wv, bv, Wo, bo, _trace=False):
    hidden_states = np.ascontiguousarray(np.asarray(hidden_states, dtype=np.float32))
    shared = {
        "Wq": np.ascontiguousarray(np.asarray(Wq, np.float32)),
        "Wk": np.ascontiguousarray(np.asarray(Wk, np.float32)),
        "Wv": np.ascontiguousarray(np.asarray(Wv, np.float32)),
        "Wo": np.ascontiguousarray(np.asarray(Wo, np.float32)),
        "bq": np.ascontiguousarray(np.asarray(bq, np.float32)),
        "bk": np.ascontiguousarray(np.asarray(bk, np.float32)),
        "bv": np.ascontiguousarray(np.asarray(bv, np.float32)),
        "bo": np.ascontiguousarray(np.asarray(bo, np.float32)),
    }
    with_bias = any(np.any(shared[b]) for b in ("bq", "bk", "bv", "bo"))
    nc = build_nc(with_bias=with_bias)
    in_maps = [{"x": hidden_states[i], **shared} for i in range(NCORES)]
    res = run_bass_kernel_spmd(
        nc, in_maps, core_ids=list(range(NCORES)), trace=_trace
    )
    out = np.stack([res.results[i]["out"] for i in range(NCORES)], axis=0)
    if _trace:
        kernel.last_results = res
    return out


if __name__ == "__main__":
    rng = np.random.default_rng(0)
    ins = {
        "hidden_states": rng.standard_normal((B, S, D), dtype=np.float32),
        **{w: (rng.standard_normal((D, D)) / np.sqrt(D)).astype(np.float32) for w in ("Wq", "Wk", "Wv", "Wo")},
        **{b: np.zeros(D, np.float32) for b in ("bq", "bk", "bv", "bo")},
    }
    o = kernel(**ins)
    print("kernel ran, out shape", o.shape)


# revision 67
# speedup vs baseline: 1.0196x; 1.0135x over previous
"""Trainium2 Bass kernel for nn_Attention_28020366639391 (sparse attention).

Math (per batch element b, reference semantics):
    q/k/v = x @ W{q,k,v} + b{q,k,v}, split into 12 heads of 64
    scores = q k^T / 8 ; rows >= 512 zeroed pre-softmax
    -> rows >= 512 have uniform probs = 1/1024 -> ctx row = mean_k(v)
    out = concat_heads(ctx) @ Wo + bo

Sharding: data-parallel on batch. 8 batch elements -> 8 NeuronCores, no
collectives. Each core gets x=[1024,768] + the full weights and computes
out=[1024,768].

Per-core dataflow (v2 — engine-balanced around the PE roofline):
    xT   [768,1024] fp32r = PE-transpose of x (d' on partitions)
    QT/KT bf16 = W{q,k}^T @ xT (Wq/Wk streamed as [768,128] column blocks,
                 2 blocks live at a time)
    V    bf16 per 128-row chunk -> Vaug [k, 12h, 64+ones] (ones column gives
                 softmax row sums for free in the ctx matmul)
    per head pair hp, key chunk kc: scores sT [kc 128, 1024] PSUM tile
                 (head 2hp in cols 0:512, head 2hp+1 in 512:1024),
                 one exp -> e[hp] bf16 [128, kc, 1024] on ACT
    ctx (heads 0-9, "ALT" layout): out [q 128, 65] bf16 matmuls
                 lhsT = e slice [k,128q], rhs = Vaug head slice [k,65];
                 col 64 = softmax denominator. Normalize via per-partition
                 reciprocal + tensor_scalar on DVE -> ctxC fp32r, then PE
                 transposes (lagged one head pair) -> TQ bf16 (d on parts).
    ctx (heads 10-11, "OLD" layout): out [65, 512] keeps d on partitions so
                 the out-projection's last contraction block needs no
                 transpose (shortens the critical tail).
    out rows 0:512   = TQ/ctxT5 blocks @ Wo(bf16)
    out rows 512:1024 = broadcast of (mean_k V) @ Wo (from column sums of x)

All projections run in fp32r (1 cycle/row on the PE); attention ctx runs in
bf16 (needed for 1 cycle/row at free-size 65). Weight tensors stay fp32r in
SBUF except Wo, which is copied to bf16 on GpSimd to match the bf16 ctx
operands (the PE cannot mix 32-bit and 16-bit operands).
"""

import os

import numpy as np

import concourse.bass as bass
import concourse.mybir as mybir
import concourse.tile as tile
from concourse import bacc
from concourse.bass_utils import run_bass_kernel_spmd
from concourse.masks import make_identity

B, S, D, H, DH = 8, 1024, 768, 12, 64
SH = 512            # active (unmasked) query rows = patches//2
DC = D // 128       # 6 chunks of the model dim
SC = S // 128       # 8 chunks of the sequence dim
HP = H // 2         # 6 head pairs
NCORES = 8
FP = mybir.dt.float32
FPR = mybir.dt.float32r
BF = mybir.dt.bfloat16
AF = mybir.ActivationFunctionType
NT = ((0, 512), (512, 256))  # free-dim tiling of a 768-wide output
ALT_HP = HP - 1     # head pairs 0..4 use the ALT (q-partition) ctx layout


def _r(ap):
    """View an fp32 AP as float32r so the PE runs at full rate."""
    return ap.bitcast(FPR)


def _body(tc, out, x, W, bvec, with_bias=True):
    nc = tc.nc
    from contextlib import ExitStack

    with ExitStack() as ctx:
        ctx.enter_context(
            nc.allow_low_precision(reason="fp32r/bf16 PE paths by design")
        )
        constp = ctx.enter_context(tc.tile_pool(name="const", bufs=1))
        wvp = ctx.enter_context(tc.tile_pool(name="wv", bufs=1))
        wop = ctx.enter_context(tc.tile_pool(name="wo", bufs=1))
        wqkp = ctx.enter_context(tc.tile_pool(name="wqk", bufs=2))
        xtp = ctx.enter_context(tc.tile_pool(name="xt", bufs=1))
        qkp = ctx.enter_context(tc.tile_pool(name="qk", bufs=1))
        vap = ctx.enter_context(tc.tile_pool(name="va", bufs=1))
        ep = ctx.enter_context(tc.tile_pool(name="e", bufs=4))
        cxc = ctx.enter_context(tc.tile_pool(name="cxc", bufs=1))
        smallp = ctx.enter_context(tc.tile_pool(name="small", bufs=1))
        op_ = ctx.enter_context(tc.tile_pool(name="o", bufs=2))
        xnp = ctx.enter_context(tc.tile_pool(name="xn", bufs=1))
        # PSUM: scores 2x[128,1024] (4 banks) + projections 1x (2 banks) +
        # two ctx accumulators (1 bank each) = 8 banks exactly
        ssp = ctx.enter_context(tc.tile_pool(name="ss", bufs=2, space="PSUM"))
        pjp = ctx.enter_context(tc.tile_pool(name="pj", bufs=1, space="PSUM"))
        cxp = ctx.enter_context(tc.tile_pool(name="cx", bufs=1, space="PSUM"))

        # ---------------- constants ----------------
        ident = constp.tile([128, 128], FP, tag="ident")
        make_identity(nc, ident[:])
        identR = constp.tile([128, 128], FPR, tag="identR")
        nc.vector.tensor_copy(identR[:], ident[:])
        onesF = constp.tile([1, 512], FP, tag="onesF")
        nc.vector.memset(onesF[:], 1.0)
        ones = constp.tile([1, 512], FPR, tag="ones")
        nc.vector.tensor_copy(ones[:], onesF[:])
        onesB = constp.tile([1, 512], BF, tag="onesB")
        nc.vector.tensor_copy(onesB[:], onesF[:])

        # ---------------- input DMAs, in schedule order ----------------
        # x first (gates everything), then Wq/Wk column blocks just-in-time
        # per head pair, Wv row blocks interleaved, Wo last.
        # x in 4 DMA instructions of 2 row-chunks each (HWDGE descriptor
        # generation serializes at ~625ns/instruction — batch aggressively)
        xn = []
        for g, (r0, nr) in enumerate(((0, 2), (2, 2), (4, 2), (6, 2))):
            t = xnp.tile([128, 2, D], FPR, tag=f"xh{g % 2}", name=f"xn{g}")
            nc.sync.dma_start(
                out=t[:, 0:nr, :],
                in_=_r(x[r0 * 128:(r0 + nr) * 128, :]).rearrange(
                    "(s p) d -> p s d", p=128
                ),
            )
            xn.append((t, r0, nr))

        wq = []
        wk = []

        def load_wqk(c):
            for nm, lst in (("Wq", wq), ("Wk", wk)):
                t = wqkp.tile([128, DC, 128], FPR, tag=nm, name=f"{nm}{c}")
                nc.sync.dma_start(
                    out=t[:],
                    in_=_r(W[nm][:, c * 128:(c + 1) * 128]).rearrange(
                        "(k p) c -> p k c", p=128
                    ),
                )
                lst.append(t)

        wv = wvp.tile([128, DC, D], FPR, tag="Wv", name="w_Wv")

        load_wqk(0)
        load_wqk(1)
        nc.sync.dma_start(
            out=wv[:], in_=_r(W["Wv"][:, :]).rearrange("(k p) d -> p k d", p=128)
        )
        load_wqk(2)
        load_wqk(3)
        load_wqk(4)
        load_wqk(5)

        # Wo: stage fp32 in the x slots, convert to bf16 on GpSimd
        wob = wop.tile([128, DC, D], BF, tag="Wob", name="w_Wob")
        for g in range(3):
            wos = xnp.tile([128, 2, D], FPR, tag=f"xh{g % 2}", name=f"wos{g}")
            nc.sync.dma_start(
                out=wos[:],
                in_=_r(W["Wo"][g * 256:(g + 1) * 256, :]).rearrange(
                    "(k p) d -> p k d", p=128
                ),
            )
            nc.gpsimd.tensor_copy(wob[:, 2 * g:2 * g + 2, :], wos[:].bitcast(FP))

        brow = {}
        if with_bias:
            for nm in ("bq", "bk", "bv", "bo"):
                t = constp.tile([1, D], FPR, tag=f"brow_{nm}", name=f"brow_{nm}")
                nc.sync.dma_start(out=t[:], in_=_r(bvec[nm][None, :]))
                brow[nm] = t
            bvT = constp.tile([128, DC], FP, tag="bvT")
            for c in range(DC):
                nc.sync.dma_start(
                    out=bvT[:, c:c + 1], in_=bvec["bv"][c * 128:(c + 1) * 128, None]
                )
            browB = constp.tile([1, D], BF, tag="browB")
            nc.vector.tensor_copy(browB[:], brow["bo"][:].bitcast(FP))

        # ---------------- persistent tensors ----------------
        xT = xtp.tile([128, DC, S], FPR, tag="xT")
        QT = qkp.tile([128, DC, SH], BF, tag="QT")
        KT = qkp.tile([128, DC, S], BF, tag="KT")
        Vaug = vap.tile([128, SC, H, DH + 1], BF, tag="Vaug")
        vones = constp.tile([128, SC * H], FP, tag="vones")
        nc.vector.memset(vones[:], 1.0)
        nc.vector.tensor_copy(
            Vaug[:, :, :, DH:DH + 1],
            vones[:].rearrange("p (k h) -> p k h", k=SC)[:, :, :, None],
        )
        # ctxC: per head pair, 512 columns interleaved as (d_local, qc) ->
        # d_local*4 + qc, so ONE xbar transpose per head pair yields
        # TQ[hp][p, qc, q] = ctxT[d = hp*128 + p, qc*128 + q]
        ctxC = cxc.tile([128, ALT_HP, 512], FPR, tag="ctxC")
        TQ = [cxc.tile([128, 4, 128], BF, tag=f"TQ{hp}", name=f"TQ{hp}")
              for hp in range(ALT_HP)]
        ctxT5 = cxc.tile([128, SH], BF, tag="ctxT5")
        xsum = constp.tile([128, DC], FPR, tag="xsum")
        mvt = constp.tile([128, DC], BF, tag="mvt")
        ttile = constp.tile([128, D], FP, tag="ttile")

        # ---------------- x transposes (PE, fp32r: 1.5 cycles/row) --------
        for t, r0, nr in xn:
            for s in range(nr):
                sc = r0 + s
                pt = ssp.tile([128, 1024], FP, tag="ss", name=f"ptx{sc}")
                for c in range(DC):
                    nc.tensor.transpose(
                        _r(pt[:, c * 128:(c + 1) * 128]),
                        t[:, s, c * 128:(c + 1) * 128],
                        identR[:],
                    )
                ptv = pt[:, 0:D].rearrange("p (c s) -> p c s", c=DC)
                nc.vector.tensor_copy(xT[:, :, sc * 128:(sc + 1) * 128], _r(ptv))

        # ------------- quantum generators (PE filler interleaving) -------
        # The exp stream on ACT (~1038ns per [128,1024] tile) is 2.4x slower
        # than the two score matmuls feeding it (~426ns). s_chunk emits
        # filler quanta of independent PE work between score pairs so the PE
        # never throttles to the ACT rate.
        from collections import deque

        fillers = deque()

        def take_filler(n):
            done = 0
            while done < n and fillers:
                try:
                    next(fillers[0])
                    done += 1
                except StopIteration:
                    fillers.popleft()

        def drain_fillers():
            while fillers:
                try:
                    next(fillers[0])
                except StopIteration:
                    fillers.popleft()

        def qkt_gen(c):
            """QT/KT projection chunk c in ~2-matmul quanta."""
            pq = pjp.tile([128, 1024], FP, tag="pj", name=f"pq{c}")
            for k in range(DC):
                nc.tensor.matmul(
                    pq[:, 0:SH],
                    wq[c][:, k, :],
                    xT[:, k, 0:SH],
                    start=(k == 0),
                    stop=(not with_bias and k == DC - 1),
                )
                if k % 2 == 1:
                    yield
            if with_bias:
                nc.tensor.matmul(
                    pq[:, 0:SH],
                    _r(brow["bq"][0:1, c * 128:(c + 1) * 128]),
                    ones[0:1, 0:SH],
                    start=False,
                    stop=True,
                )
            nc.vector.tensor_copy(QT[:, c, :], pq[:, 0:SH])
            yield
            pk = pjp.tile([128, 1024], FP, tag="pj", name=f"pk{c}")
            for sg in range(2):
                for k in range(DC):
                    nc.tensor.matmul(
                        pk[:, sg * 512:sg * 512 + 512],
                        wk[c][:, k, :],
                        xT[:, k, sg * 512:sg * 512 + 512],
                        start=(k == 0),
                        stop=(not with_bias and k == DC - 1),
                    )
                    if k % 2 == 1:
                        yield
                if with_bias:
                    nc.tensor.matmul(
                        pk[:, sg * 512:sg * 512 + 512],
                        _r(brow["bk"][0:1, c * 128:(c + 1) * 128]),
                        ones[0:1, 0:512],
                        start=False,
                        stop=True,
                    )
                nc.vector.tensor_copy(
                    KT[:, c, sg * 512:sg * 512 + 512],
                    pk[:, sg * 512:sg * 512 + 512],
                )
                yield

        def v_gen(sc):
            """V projection chunk sc in ~2-matmul quanta."""
            pv = pjp.tile([128, 1024], FP, tag="pj", name=f"pv{sc}")
            for n0, nw in NT:
                for k in range(DC):
                    nc.tensor.matmul(
                        pv[:, n0:n0 + nw],
                        xT[:, k, sc * 128:(sc + 1) * 128],
                        wv[:, k, n0:n0 + nw],
                        start=(k == 0),
                        stop=(not with_bias and k == DC - 1),
                    )
                    if k % 2 == 1:
                        yield
                if with_bias:
                    nc.tensor.matmul(
                        pv[:, n0:n0 + nw],
                        ones[0:1, 0:128],
                        _r(brow["bv"][0:1, n0:n0 + nw]),
                        start=False,
                        stop=True,
                    )
            nc.vector.tensor_copy(
                Vaug[:, sc, :, 0:DH],
                pv[:, 0:D].rearrange("p (h e) -> p h e", h=H),
            )
            yield

        def s_chunk(hp, take=2, must=None):
            """Scores + exp for head pair hp; returns the e tile.

            `must` is the generator producing this head pair's QT/KT — it is
            drained first so every score matmul reads completed copies
            (emission order defines the dataflow graph)."""
            if must is not None:
                for _ in must:
                    pass
            e = ep.tile([128, SC, 1024], BF, tag="e", name=f"e{hp}")
            for kc in range(SC):
                st = ssp.tile([128, 1024], FP, tag="ss", name=f"st{hp}_{kc}")
                for j in range(2):
                    off = j * DH
                    nc.tensor.matmul(
                        st[:, j * 512:j * 512 + 512],
                        KT[off:off + DH, hp, kc * 128:(kc + 1) * 128],
                        QT[off:off + DH, hp, :],
                        start=True,
                        stop=True,
                        tile_position=(off, 0),
                    )
                nc.scalar.activation(e[:, kc, :], st[:], AF.Exp, scale=0.125)
                take_filler(take)
            return e

        def tq_part(hp):
            """PE-transpose ctxC[hp] -> TQ[hp] (the xbar DMA transpose's
            completion semaphore is unreliable for PE consumers in real
            execution, so stay on the well-trodden matmul path)."""
            ptq = pjp.tile([128, 1024], FP, tag="pj", name=f"ptq{hp}")
            for qc in range(4):
                nc.tensor.transpose(
                    _r(ptq[:, qc * 128:(qc + 1) * 128]),
                    ctxC[:, hp, qc * 128:(qc + 1) * 128],
                    identR[:],
                )
            nc.vector.tensor_copy(
                TQ[hp][:].rearrange("p a b -> p (a b)"), _r(ptq[:, 0:512])
            )

        def tq_gen(hp):
            tq_part(hp)
            yield

        def ctx_gen(hp, e):
            """ctx_norm as filler quanta (for head pairs that only gate a
            LATER s_chunk's e-buffer, not the one they interleave into)."""
            rec = smallp.tile([128, 2, 4], FP, tag="rec", name=f"rec{hp}")
            for j, eng in ((0, nc.vector), (1, nc.vector)):
                pc = cxp.tile([128, 4, DH + 1], FP, tag=f"cx{j}", name=f"cx{hp}_{j}")
                h = 2 * hp + j
                for qc in range(4):
                    for kc in range(SC):
                        nc.tensor.matmul(
                            pc[:, qc, :],
                            e[:, kc, j * 512 + qc * 128:j * 512 + qc * 128 + 128],
                            Vaug[:, kc, h, :],
                            start=(kc == 0),
                            stop=(kc == SC - 1),
                        )
                    if qc == 1:
                        yield
                nc.vector.reciprocal(rec[:, j, :], pc[:, :, DH:DH + 1])
                for qc in range(4):
                    eng.tensor_scalar_mul(
                        ctxC[:, hp, qc * 128 + j * DH:qc * 128 + (j + 1) * DH],
                        pc[:, qc, 0:DH],
                        rec[:, j, qc:qc + 1],
                    )
                yield

        def ctx_norm(hp, e):
            """ALT-layout ctx + normalization for head pair hp."""
            rec = smallp.tile([128, 2, 4], FP, tag="rec", name=f"rec{hp}")
            for j, eng in ((0, nc.vector), (1, nc.vector)):
                pc = cxp.tile([128, 4, DH + 1], FP, tag=f"cx{j}", name=f"cx{hp}_{j}")
                h = 2 * hp + j
                for qc in range(4):
                    for kc in range(SC):
                        nc.tensor.matmul(
                            pc[:, qc, :],
                            e[:, kc, j * 512 + qc * 128:j * 512 + qc * 128 + 128],
                            Vaug[:, kc, h, :],
                            start=(kc == 0),
                            stop=(kc == SC - 1),
                        )
                nc.vector.reciprocal(rec[:, j, :], pc[:, :, DH:DH + 1])
                for qc in range(4):
                    # ctxC col layout qc*128 + d_local: the xbar transpose's
                    # 3D out [128, 4, 128] maps out[p, f1, f2] = in[f2, f1*128+p]
                    eng.tensor_scalar_mul(
                        ctxC[:, hp, qc * 128 + j * DH:qc * 128 + (j + 1) * DH],
                        pc[:, qc, 0:DH],
                        rec[:, j, qc:qc + 1],
                    )

        def meanv_gen():
            """Masked-row tail: mean_k(V) @ Wo + broadcast rows 512:1024."""
            pm = pjp.tile([128, 1024], FP, tag="pj", name="pm")
            for c in range(DC):
                for k in range(DC):
                    nc.tensor.matmul(
                        pm[:, c:c + 1],
                        wv[:, k, c * 128:(c + 1) * 128].bitcast(FP),
                        xsum[:, k:k + 1].bitcast(FP),

                        start=(k == 0),
                        stop=(k == DC - 1),
                    )
                if c % 2 == 1:
                    yield
            if with_bias:
                nc.vector.scalar_tensor_tensor(
                    mvt[:], pm[:, 0:DC], 1.0 / S,
                    bvT[:], op0=mybir.AluOpType.mult, op1=mybir.AluOpType.add,
                )
            else:
                nc.vector.tensor_scalar_mul(mvt[:], pm[:, 0:DC], 1.0 / S)
            yield
            pt2 = pjp.tile([128, 1024], FP, tag="pj", name="pt2")
            for n0, nw in NT:
                for k in range(DC):
                    nc.tensor.matmul(
                        pt2[0:1, n0:n0 + nw],
                        mvt[:, k:k + 1],
                        wob[:, k, n0:n0 + nw],
                        start=(k == 0),
                        stop=(not with_bias and k == DC - 1),
                    )
                    if k % 2 == 1:
                        yield
                if with_bias:
                    nc.tensor.matmul(
                        pt2[0:1, n0:n0 + nw],
                        onesB[0:1, 0:1],
                        browB[0:1, n0:n0 + nw],
                        start=False,
                        stop=True,
                    )
            trow = constp.tile([1, D], FP, tag="trow")
            nc.vector.tensor_copy(trow[:], pt2[0:1, 0:D])
            nc.gpsimd.partition_broadcast(ttile[:], trow[0:1, :])
            for sc in range(SH // 128, SC):
                nc.sync.dma_start(
                    out=out[sc * 128:(sc + 1) * 128, :], in_=ttile[:]
                )
            yield

        def ctx_old(hp, e):
            """OLD-layout ctx for the last head pair: ctxT5 [128 d, 512 q].
            Both heads share one PSUM tile (regions 0:512 / 512:1024)."""
            p5 = pjp.tile([128, 1024], FP, tag="pj", name="p5")
            for j in range(2):
                h = 2 * hp + j
                for kc in range(SC):
                    nc.tensor.matmul(
                        p5[0:DH + 1, j * 512:j * 512 + 512],
                        Vaug[:, kc, h, :],
                        e[:, kc, j * 512:j * 512 + 512],
                        start=(kc == 0),
                        stop=(kc == SC - 1),
                    )
            for j in range(2):
                recrow = smallp.tile([1, SH], FP, tag="recrow", name=f"rr{j}")
                nc.vector.reciprocal(recrow[:], p5[DH:DH + 1, j * 512:j * 512 + 512])
                bsb = smallp.tile([DH, SH], FP, tag="bsb", name=f"bsb{j}")
                nc.gpsimd.partition_broadcast(bsb[:], recrow[0:1, :])
                nc.vector.tensor_mul(
                    ctxT5[j * DH:(j + 1) * DH, :],
                    p5[0:DH, j * 512:j * 512 + 512],
                    bsb[:],
                )

        # ---------------- main schedule ----------------
        # drain one generator fully
        def run_all(g):
            for _ in g:
                pass

        run_all(qkt_gen(0))
        g = [qkt_gen(c) for c in range(1, DC)]
        fillers.append(g[0])
        es = [s_chunk(0, take=4)]
        fillers.append(g[1])
        fillers.append(v_gen(0))
        fillers.append(v_gen(1))
        es.append(s_chunk(1, take=4, must=g[0]))
        fillers.append(v_gen(2))
        fillers.append(v_gen(3))
        fillers.append(v_gen(4))
        fillers.append(g[2])
        es.append(s_chunk(2, take=4, must=g[1]))
        fillers.append(v_gen(5))
        fillers.append(v_gen(6))
        fillers.append(v_gen(7))
        fillers.append(g[3])
        fillers.append(ctx_gen(0, es[0]))
        es.append(s_chunk(3, take=4, must=g[2]))
        drain_fillers()
        fillers.append(g[4])
        fillers.append(ctx_gen(1, es[1]))
        fillers.append(tq_gen(0))
        fillers.append(ctx_gen(2, es[2]))
        fillers.append(tq_gen(1))
        es.append(s_chunk(4, take=4, must=g[3]))
        drain_fillers()
        # column sums of x for the masked-row tail (after the qkt5 copies so
        # it does not block them on the in-order DVE queue)
        for c in range(DC):
            nc.vector.tensor_reduce(
                xsum[:, c:c + 1, None], xT[:, c, :].bitcast(FP),
                axis=mybir.AxisListType.X, op=mybir.AluOpType.add,
            )
        fillers.append(meanv_gen())
        fillers.append(ctx_gen(3, es[3]))
        fillers.append(tq_gen(2))
        fillers.append(ctx_gen(4, es[4]))
        fillers.append(tq_gen(3))
        es.append(s_chunk(5, take=4, must=g[4]))
        drain_fillers()
        tq_part(4)
        # last head pair in the OLD layout (no transpose on the tail)
        ctx_old(ALT_HP, es[ALT_HP])

        # ---------------- output projection, rows 0:512 ----------------
        # qc 0/1: accumulate the TQ blocks (dc 0..4) before the exp-gated
        # ctx_old, close with the ctxT5 block after; qc 2/3 run whole
        for qc in range(4):
            po = ssp.tile([128, 1024], FP, tag="ss", name=f"po{qc}")
            osb = op_.tile([128, D], FP, tag="osb", name=f"osb{qc}")
            for n0, nw in NT:
                for dc in range(ALT_HP):
                    nc.tensor.matmul(
                        po[:, n0:n0 + nw],
                        TQ[dc][:, qc, :],
                        wob[:, dc, n0:n0 + nw],
                        start=(dc == 0),
                        stop=False,
                    )
                nc.tensor.matmul(
                    po[:, n0:n0 + nw],
                    ctxT5[:, qc * 128:(qc + 1) * 128],
                    wob[:, ALT_HP, n0:n0 + nw],
                    start=False,
                    stop=(not with_bias),
                )
                if with_bias:
                    nc.tensor.matmul(
                        po[:, n0:n0 + nw],
                        onesB[0:1, 0:128],
                        browB[0:1, n0:n0 + nw],
                        start=False,
                        stop=True,
                    )
                nc.vector.tensor_copy(osb[:, n0:n0 + nw], po[:, n0:n0 + nw])
                nc.sync.dma_start(
                    out=out[qc * 128:(qc + 1) * 128, n0:n0 + nw],
                    in_=osb[:, n0:n0 + nw],
                )

def build_nc(with_bias=True):
    nc = bacc.Bacc("TRN2", target_bir_lowering=False, debug=False, num_devices=NCORES)
    x = nc.dram_tensor("x", [S, D], FP, kind="ExternalInput").ap()
    W = {
        nm: nc.dram_tensor(nm, [D, D], FP, kind="ExternalInput").ap()
        for nm in ("Wq", "Wk", "Wv", "Wo")
    }
    bvec = {
        nm: nc.dram_tensor(nm, [D], FP, kind="ExternalInput").ap()
        for nm in ("bq", "bk", "bv", "bo")
    }
    out = nc.dram_tensor("out", [S, D], FP, kind="ExternalOutput").ap()
    with tile.TileContext(nc) as tc:
        _body(tc, out, x, W, bvec, with_bias=with_bias)
    nc.compile()
    return nc


def kernel(hidden_states, Wq, bq, Wk, bk, Wv, bv, Wo, bo, _trace=False):
    hidden_states = np.ascontiguousarray(np.asarray(hidden_states, dtype=np.float32))
    shared = {
        "Wq": np.ascontiguousarray(np.asarray(Wq, np.float32)),
        "Wk": np.ascontiguousarray(np.asarray(Wk, np.float32)),
        "Wv": np.ascontiguousarray(np.asarray(Wv, np.float32)),
        "Wo": np.ascontiguousarray(np.asarray(Wo, np.float32)),
        "bq": np.ascontiguousarray(np.asarray(bq, np.float32)),
        "bk": np.ascontiguousarray(np.asarray(bk, np.float32)),
        "bv": np.ascontiguousarray(np.asarray(bv, np.float32)),
        "bo": np.ascontiguousarray(np.asarray(bo, np.float32)),
    }
    with_bias = any(np.any(shared[b]) for b in ("bq", "bk", "bv", "bo"))
    nc = build_nc(with_bias=with_bias)
    in_maps = [{"x": hidden_states[i], **shared} for i in range(NCORES)]
    res = run_bass_kernel_spmd(
        nc, in_maps, core_ids=list(range(NCORES)), trace=_trace
    )
    out = np.stack([res.results[i]["out"] for i in range(NCORES)], axis=0)
    if _trace:
        kernel.last_results = res
    return out


if __name__ == "__main__":
    rng = np.random.default_rng(0)
    ins = {
        "hidden_states": rng.standard_normal((B, S, D), dtype=np.float32),
        **{w: (rng.standard_normal((D, D)) / np.sqrt(D)).astype(np.float32) for w in ("Wq", "Wk", "Wv", "Wo")},
        **{b: np.zeros(D, np.float32) for b in ("bq", "bk", "bv", "bo")},
    }
    o = kernel(**ins)
    print("kernel ran, out shape", o.shape)
